# revision 36
# baseline (speedup 1.0000x reference)
"""Trainium2 Bass kernel for a ViT attention block (LN -> QKV -> RoPE -> attn -> out-proj).

Sharding: 8 cores = 2 batches x 4 head-groups (4 heads each). Each core computes
a partial out-projection (its 4 heads) for one batch, transposed as [D, N].
Host sums the 4 partials per batch and transposes back. LayerNorm gamma/beta are
folded into the QKV weights/bias on the host.

Device layout (per core):
  - LN in [tok, d] layout (bn_stats per-partition), normalized output cast bf16,
    PE-transposed into xT tiles ([d-chunk, tok] layout, split into 5 token
    groups so the QKV projection can start before LayerNorm finishes).
  - Q,K projected directly into [feat, tok] (lhsT = weight chunks); V projected
    into [tok, feat] (lhsT = xT chunks). QKV bias enters as a K=1 matmul row.
  - RoPE in [feat, tok] with host-built cos/sin tables ([128] rows = 2 heads
    stacked; cls token and padding encoded in the tables). The rotate-half
    partner comes from partition-shifting SBUF->SBUF DMAs.
  - Attention transposed: dotsT[k,q] = matmul(lhsT=kT, rhs=qT), exp on ACT
    (logits are bounded, no max subtraction), attn@v as outT[65,q] with an
    appended ones column in V giving the softmax denominator for free.
  - Denominators are DMA'd into 32-aligned partitions of one tile so a single
    full-width reciprocal+cast serves all 4 heads of a q-chunk; normalize via
    ones outer-product broadcast (PE) + DVE multiply; out-proj accumulates the
    256 head dims; result DMA'd out as [D, N] (transposed).
All matmuls bf16 with f32 PSUM accumulation.
"""

import sys

sys.path.insert(0, "/opt/trn_rl_repo")

import numpy as np
import ml_dtypes

import concourse.bacc as bacc
import concourse.mybir as mybir
import concourse.tile as tile
from concourse.bass_utils import run_bass_kernel_spmd

F32 = mybir.dt.float32
BF16 = mybir.dt.bfloat16
AF = mybir.ActivationFunctionType
OP = mybir.AluOpType
BF = ml_dtypes.bfloat16

B, N, D = 2, 2049, 1024
DH = 64
HPC = 4  # heads per core
NT = 17  # 128-token tiles (padded to 2176)
TPAD = NT * 128
SCALE = DH ** -0.5
# q-column chunks [offset, width]; the tail chunk is the single real token 2048
JJ = [(2048, 1), (0, 1024), (1024, 1024)]
J5 = [(0, 512), (512, 512), (1024, 512), (1536, 512), (2048, 128)]
# token groups backing the 5 xT tiles (4+4+4+4+1 of the 17 token tiles)
TG = [(0, 512), (512, 512), (1024, 512), (1536, 512), (2048, 128)]


def _subs(jw):
    return [(s, min(512, jw - s)) for s in range(0, jw, 512)]


def _tg_of(col):
    return min(col // 512, 4)


def _build():
    nc = bacc.Bacc("TRN2", target_bir_lowering=False, debug=False, num_devices=8)

    x_d = nc.declare_dram_parameter("x", [N, D], F32, False)
    wqk_d = nc.declare_dram_parameter("wqk", [D, 512], BF16, False)
    wv_d = nc.declare_dram_parameter("wv", [D, 256], BF16, False)
    wo_d = nc.declare_dram_parameter("wo", [256, D], BF16, False)
    bqk_d = nc.declare_dram_parameter("bqk", [1, 512], BF16, False)
    bv_d = nc.declare_dram_parameter("bv", [1, 256], BF16, False)
    cos_d = nc.declare_dram_parameter("cos2", [128, TPAD], BF16, False)
    sin_d = nc.declare_dram_parameter("sinf2", [128, TPAD], BF16, False)
    idn_d = nc.declare_dram_parameter("ident", [128, 128], BF16, False)
    out_d = nc.declare_dram_parameter("out", [D, N], F32, True)

    with tile.TileContext(nc) as tc:
        with (
            tc.tile_pool(name="const", bufs=1) as cpool,
            tc.tile_pool(name="persist", bufs=1) as ppool,
            tc.tile_pool(name="work", bufs=2) as wpool,
            tc.tile_pool(name="psum", bufs=2, space="PSUM") as pspool,
        ):
            # ---------------- constants ----------------
            wqk_sb = [cpool.tile([128, 512], BF16, tag=f"wqk{c}", name=f"wqk{c}") for c in range(8)]
            wv_sb = [cpool.tile([128, 256], BF16, tag=f"wv{c}", name=f"wv{c}") for c in range(8)]
            wo_sb = [cpool.tile([128, 1024], BF16, tag=f"wo{c}", name=f"wo{c}") for c in range(2)]
            bqk_sb = cpool.tile([1, 512], BF16, tag="bqk", name="bqk")
            bv_sb = cpool.tile([1, 256], BF16, tag="bv", name="bv")
            cos_sb = cpool.tile([128, TPAD], BF16, tag="cos", name="cos")
            sin_sb = cpool.tile([128, TPAD], BF16, tag="sin", name="sin")
            idn_sb = cpool.tile([128, 128], BF16, tag="idn", name="idn")
            ones_sb = cpool.tile([1, TPAD], BF16, tag="ones", name="ones")

            nc.sync.dma_start(out=idn_sb[:], in_=idn_d[:])

            def _load_weights():
                for c in range(8):
                    nc.sync.dma_start(out=wqk_sb[c][:], in_=wqk_d[c * 128:(c + 1) * 128, :])
                    nc.sync.dma_start(out=wv_sb[c][:], in_=wv_d[c * 128:(c + 1) * 128, :])

            def _load_consts():
                for c in range(2):
                    nc.gpsimd.dma_start(out=wo_sb[c][:], in_=wo_d[c * 128:(c + 1) * 128, :])
                nc.gpsimd.dma_start(out=bqk_sb[:], in_=bqk_d[:])
                nc.gpsimd.dma_start(out=bv_sb[:], in_=bv_d[:])
                nc.gpsimd.dma_start(out=cos_sb[:], in_=cos_d[:])
                nc.gpsimd.dma_start(out=sin_sb[:], in_=sin_d[:])

            nc.vector.memset(ones_sb[:], 1.0)
            eps_sb = cpool.tile([128, 1], F32, tag="eps", name="eps")
            nc.vector.memset(eps_sb[:], 1e-5)
            ones64_sb = cpool.tile([128, 64], BF16, tag="ones64", name="ones64")
            nc.vector.memset(ones64_sb[:], 1.0)

            # ---------------- persistent activations ----------------
            # xT[g]: [128, 8 dchunks, tg-width] per token group g (fine-grained deps)
            xTg = [
                ppool.tile([128, 8 * tw], BF16, tag=f"xT{g}", name=f"xT{g}")
                for g, (to, tw) in enumerate(TG)
            ]
            xT3 = [
                xTg[g][:, :].rearrange("p (c t) -> p c t", c=8) for g in range(5)
            ]

            def xslice(c, jo, jw):
                g = _tg_of(jo)
                to, tw = TG[g]
                assert jo + jw <= to + tw
                return xT3[g][:, c, jo - to:jo - to + jw]

            # qkT tiles: 0,1 = q head-pairs (h01, h23); 2,3 = k head-pairs
            qkT_sb = [ppool.tile([128, TPAD], BF16, tag=f"qkT{f}", name=f"qkT{f}") for f in range(4)]
            vaug_sb = [ppool.tile([128, 260], BF16, tag=f"v{k}", name=f"v{k}") for k in range(NT)]

            # ---------------- phase A: LayerNorm + transpose ----------------
            for i in range(NT):
                xa = wpool.tile([128, D], F32, tag="xa", name="xa", bufs=3)
                if i < 16:
                    nc.sync.dma_start(out=xa[:], in_=x_d[i * 128:(i + 1) * 128, :])
                else:
                    nc.vector.memset(xa[:], 0.0)
                    nc.sync.dma_start(out=xa[0:1, :], in_=x_d[2048:2049, :])
                if i == 2:
                    _load_weights()
                if i == 4:
                    _load_consts()
                stats = wpool.tile([128, 12], F32, tag="stats", name="stats", bufs=3)
                mv = wpool.tile([128, 2], F32, tag="mv", name="mv", bufs=4)
                nc.vector.bn_stats(stats[:, 0:6], xa[:, 0:512])
                nc.vector.bn_stats(stats[:, 6:12], xa[:, 512:1024])
                nc.vector.bn_aggr(mv[:], stats[:])
                std = wpool.tile([128, 1], F32, tag="std", name="std", bufs=4)
                rstd = wpool.tile([128, 1], F32, tag="rstd", name="rstd", bufs=4)
                murstd = wpool.tile([128, 1], F32, tag="murstd", name="murstd")
                nc.scalar.activation(std[:], mv[:, 1:2], AF.Sqrt, bias=eps_sb[:])
                nc.vector.reciprocal(rstd[:], std[:])
                nc.vector.tensor_mul(murstd[:], mv[:, 0:1], rstd[:])
                xn = wpool.tile([128, D], BF16, tag="xn", name="xn", bufs=3)
                nc.vector.tensor_scalar(
                    xn[:], xa[:], rstd[:], murstd[:], OP.mult, OP.subtract
                )
                g = _tg_of(i * 128)
                to, tw = TG[g]
                for s in range(2):
                    pst = pspool.tile([128, 512], BF16, tag="dots", name="pst", bufs=3)
                    for c in range(4):
                        nc.tensor.transpose(
                            pst[:, c * 128:(c + 1) * 128],
                            xn[:, (4 * s + c) * 128:(4 * s + c + 1) * 128],
                            idn_sb[:],
                        )
                    nc.vector.tensor_copy(
                        xT3[g][:, 4 * s:4 * s + 4, i * 128 - to:(i + 1) * 128 - to],
                        pst[:, :].rearrange("p (c t) -> p c t", c=4),
                    )

            # ---------------- phase B1: Q,K projection + RoPE ----------------
            for f in (0, 2, 1, 3):
                qf = wpool.tile([128, TPAD], F32, tag="qf", name="qf", bufs=2)
                for jo, jw in J5:
                    psq = pspool.tile([128, 512], F32, tag="po", name="psq", bufs=2)
                    for c in range(8):
                        nc.tensor.matmul(
                            psq[:, 0:jw],
                            wqk_sb[c][:, f * 128:(f + 1) * 128],
                            xslice(c, jo, jw),
                            start=(c == 0),
                            stop=False,
                        )
                    nc.tensor.matmul(
                        psq[:, 0:jw],
                        bqk_sb[:, f * 128:(f + 1) * 128],
                        ones_sb[:, jo:jo + jw],
                        start=False,
                        stop=True,
                    )
                    nc.scalar.copy(qf[:, jo:jo + jw], psq[:, 0:jw])
                # rotate-half partner via partition-shifting DMAs
                qs = wpool.tile([128, TPAD], F32, tag="qs", name="qs", bufs=1)
                nc.sync.dma_start(out=qs[0:32, :], in_=qf[32:64, :])
                nc.sync.dma_start(out=qs[32:64, :], in_=qf[0:32, :])
                nc.sync.dma_start(out=qs[64:96, :], in_=qf[96:128, :])
                nc.sync.dma_start(out=qs[96:128, :], in_=qf[64:96, :])
                for jo, jw in J5:
                    t1 = wpool.tile([128, 512], F32, tag="t1", name="t1", bufs=1)
                    t2 = wpool.tile([128, 512], F32, tag="t2", name="t2", bufs=1)
                    nc.vector.tensor_mul(
                        t1[:, 0:jw], qf[:, jo:jo + jw], cos_sb[:, jo:jo + jw]
                    )
                    nc.vector.tensor_mul(
                        t2[:, 0:jw], qs[:, jo:jo + jw], sin_sb[:, jo:jo + jw]
                    )
                    nc.vector.tensor_add(
                        qkT_sb[f][:, jo:jo + jw], t1[:, 0:jw], t2[:, 0:jw]
                    )

            # ---------------- phase B2: V projection -> v_aug ----------------
            for k in range(NT):
                psv = pspool.tile([128, 256], F32, tag="po", name="psv", bufs=2)
                for c in range(8):
                    nc.tensor.matmul(
                        psv[:],
                        xslice(c, k * 128, 128),
                        wv_sb[c][:],
                        start=(c == 0),
                        stop=False,
                    )
                nc.tensor.matmul(
                    psv[:],
                    ones_sb[:, k * 128:(k + 1) * 128],
                    bv_sb[:],
                    start=False,
                    stop=True,
                )
                va = vaug_sb[k]
                va3 = va[:, :].rearrange("p (a b) -> p a b", a=4)
                if k < 16:
                    nc.scalar.copy(
                        va3[:, :, 0:64], psv[:, :].rearrange("p (a b) -> p a b", a=4)
                    )
                    nc.vector.memset(va3[:, :, 64:65], 1.0)
                else:
                    # only token 2048 is real; zero rows kill padded keys
                    nc.vector.memset(va[:], 0.0)
                    va3r = va[0:1, :].rearrange("p (a b) -> p a b", a=4)
                    nc.scalar.copy(
                        va3r[:, :, 0:64],
                        psv[0:1, :].rearrange("p (a b) -> p a b", a=4),
                    )
                    nc.vector.memset(va3r[:, :, 64:65], 1.0)

            # ---------------- phases C+D: attention, normalize, out-proj ----------------
            # The epilogue (normalize + out-proj) of chunk j is emitted AFTER
            # chunk j+1's attention stream: the PE queue is in-order, so an
            # epilogue waiting on the reciprocal chain would otherwise block
            # the next chunk's dots and starve ACT.

            def attention(jo, jw):
                oev_l = []
                dnp = [
                    wpool.tile([64, 1024], BF16, tag=f"dnp{p}", name=f"dnp{p}", bufs=2)
                    for p in range(2)
                ]
                for h in range(HPC):
                    qt = qkT_sb[h // 2]
                    ktile = qkT_sb[2 + h // 2]
                    po = (h % 2) * 64
                    pso = [
                        pspool.tile([65, 512], F32, tag="po", name=f"pso{si}", bufs=2)
                        for si in range(2 if jw > 1 else 1)
                    ]
                    if jw == 1:
                        psd = pspool.tile([128, NT], F32, tag="dots", name="psdt", bufs=3)
                        for k in range(NT):
                            nc.tensor.matmul(
                                psd[:, k:k + 1],
                                ktile[po:po + 64, k * 128:(k + 1) * 128],
                                qt[po:po + 64, jo:jo + 1],
                            )
                        ex = wpool.tile([128, 1024], BF16, tag="ex", name="ex", bufs=4)
                        nc.scalar.activation(ex[:, 0:NT], psd[:], AF.Exp, scale=SCALE)
                        for k in range(NT):
                            nc.tensor.matmul(
                                pso[0][:, 0:1],
                                vaug_sb[k][:, h * 65:h * 65 + 65],
                                ex[:, k:k + 1],
                                start=(k == 0),
                                stop=(k == NT - 1),
                                skip_group_check=True,
                            )
                    else:
                        for k in range(NT):
                            psd = pspool.tile([128, 1024], F32, tag="dots", name="psd", bufs=3)
                            for so, sw in _subs(jw):
                                nc.tensor.matmul(
                                    psd[:, so:so + sw],
                                    ktile[po:po + 64, k * 128:(k + 1) * 128],
                                    qt[po:po + 64, jo + so:jo + so + sw],
                                )
                            ex = wpool.tile([128, 1024], BF16, tag="ex", name="ex", bufs=4)
                            nc.scalar.activation(
                                ex[:, 0:jw], psd[:, 0:jw], AF.Exp, scale=SCALE
                            )
                            for si, (so, sw) in enumerate(_subs(jw)):
                                nc.tensor.matmul(
                                    pso[si][:, 0:sw],
                                    vaug_sb[k][:, h * 65:h * 65 + 65],
                                    ex[:, so:so + sw],
                                    start=(k == 0),
                                    stop=(k == NT - 1),
                                    skip_group_check=True,
                                )
                    # evacuate quickly to free the PSUM slots
                    oev = (wpool.tile([65, 64], BF16, tag="oevt", name="oevt", bufs=4)
                           if jw == 1 else
                           wpool.tile([65, 1024], BF16, tag="oev", name="oev", bufs=8))
                    for si, (so, sw) in enumerate(_subs(jw)):
                        nc.vector.tensor_copy(oev[:, so:so + sw], pso[si][:, 0:sw])
                    oev_l.append(oev)
                    # denominator row -> 32-aligned partition of the packed tile
                    nc.gpsimd.dma_start(
                        out=dnp[h // 2][32 * (h % 2):32 * (h % 2) + 1, 0:jw],
                        in_=oev[64:65, 0:jw],
                    )
                return oev_l, dnp

            def epilogue(jo, jw, oev_l, dnp):
                an = [
                    wpool.tile([128, 1024], BF16, tag=f"an{ch}", name=f"an{ch}", bufs=2)
                    for ch in range(2)
                ]
                dnb = [
                    wpool.tile([64, 1024], BF16, tag=f"dnb{p}", name=f"dnb{p}", bufs=2)
                    for p in range(2)
                ]
                for p in range(2):
                    for so, sw in _subs(jw):
                        dnr = wpool.tile([64, 512], F32, tag="dnr", name="dnr", bufs=2)
                        nc.vector.reciprocal(dnr[:, 0:sw], dnp[p][:, so:so + sw])
                        nc.vector.tensor_copy(dnb[p][:, so:so + sw], dnr[:, 0:sw])
                for h in range(HPC):
                    for so, sw in _subs(jw):
                        psb = pspool.tile([64, 512], F32, tag="po", name="psb", bufs=2)
                        hp, hr = h // 2, 32 * (h % 2)
                        nc.tensor.matmul(
                            psb[:, 0:sw],
                            ones64_sb[hr:hr + 1, :],
                            dnb[hp][hr:hr + 1, so:so + sw],
                        )
                        nt = wpool.tile([64, 512], BF16, tag="nt", name="nt", bufs=2)
                        nc.vector.tensor_mul(
                            nt[:, 0:sw], oev_l[h][0:64, so:so + sw], psb[:, 0:sw]
                        )
                        nc.gpsimd.dma_start(
                            out=an[h // 2][(h % 2) * 64:(h % 2) * 64 + 64, so:so + sw],
                            in_=nt[:, 0:sw],
                        )
                for of in range(8):
                    for so, sw in _subs(jw):
                        swr = min(sw, max(0, N - (jo + so)))
                        pp = pspool.tile([128, 512], F32, tag="po", name="pp", bufs=2)
                        nc.tensor.matmul(
                            pp[:, 0:sw],
                            wo_sb[0][:, of * 128:(of + 1) * 128],
                            an[0][:, so:so + sw],
                            start=True,
                            stop=False,
                        )
                        nc.tensor.matmul(
                            pp[:, 0:sw],
                            wo_sb[1][:, of * 128:(of + 1) * 128],
                            an[1][:, so:so + sw],
                            start=False,
                            stop=True,
                        )
                        oo = wpool.tile([128, 512], F32, tag="oo", name="oo", bufs=2)
                        nc.vector.tensor_copy(oo[:, 0:swr], pp[:, 0:swr])
                        nc.sync.dma_start(
                            out=out_d[of * 128:(of + 1) * 128, jo + so:jo + so + swr],
                            in_=oo[:, 0:swr],
                        )

            prev = None
            for jo, jw in JJ:
                cur = attention(jo, jw)
                if prev is not None:
                    epilogue(*prev)
                prev = (jo, jw) + cur
            epilogue(*prev)
    nc.compile()
    return nc


_NC = None


def _get_nc():
    global _NC
    if _NC is None:
        _NC = _build()
    return _NC


def _make_inputs(x, ln_gamma, ln_beta, w_qkv, w_out):
    w_eff = (w_qkv * ln_gamma[:, None].astype(np.float32)).astype(np.float32)
    b_all = (ln_beta.astype(np.float32) @ w_qkv.astype(np.float32)).astype(np.float32)

    inv = 1.0 / (10000.0 ** (np.arange(0, 64, 2, dtype=np.float64) / 64.0))
    fr = np.arange(2048, dtype=np.float64)[:, None] * inv[None, :]
    cos64 = np.concatenate([np.cos(fr), np.cos(fr)], axis=1).T  # [64, 2048]
    sinf64 = np.concatenate([-np.sin(fr), np.sin(fr)], axis=1).T
    cos2 = np.ones((128, TPAD), np.float32)
    sinf2 = np.zeros((128, TPAD), np.float32)
    cos2[:, 1:2049] = np.tile(cos64, (2, 1)).astype(np.float32)
    sinf2[:, 1:2049] = np.tile(sinf64, (2, 1)).astype(np.float32)
    ident = np.eye(128, dtype=BF)

    in_maps = []
    for c in range(8):
        b, g = c // 4, c % 4
        cols = slice(256 * g, 256 * g + 256)
        wqk = np.concatenate(
            [w_eff[:, 0:1024][:, cols], w_eff[:, 1024:2048][:, cols]], axis=1
        ).astype(BF)
        wv = w_eff[:, 2048:3072][:, cols].astype(BF)
        wo = w_out[cols, :].astype(BF)
        bqk = np.concatenate([b_all[0:1024][cols], b_all[1024:2048][cols]])[None, :].astype(BF)
        bv = b_all[2048:3072][cols][None, :].astype(BF)
        in_maps.append(
            {
                "x": np.ascontiguousarray(x[b]).astype(np.float32),
                "wqk": np.ascontiguousarray(wqk),
                "wv": np.ascontiguousarray(wv),
                "wo": np.ascontiguousarray(wo),
                "bqk": bqk,
                "bv": bv,
                "cos2": cos2.astype(BF),
                "sinf2": sinf2.astype(BF),
                "ident": ident,
            }
        )
    return in_maps


def kernel(x, ln_gamma, ln_beta, w_qkv, w_out, _trace=False, _trace_kwargs=None):
    nc = _get_nc()
    in_maps = _make_inputs(x, ln_gamma, ln_beta, w_qkv, w_out)
    res = run_bass_kernel_spmd(
        nc, in_maps, core_ids=list(range(8)), trace=_trace,
        **(_trace_kwargs or {}),
    )
    out = np.zeros((B, N, D), np.float32)
    for c in range(8):
        out[c // 4] += np.asarray(res.results[c]["out"], np.float32).T
    if _trace:
        return out, res
    return out


# revision 37
# speedup vs baseline: 1.1575x; 1.1575x over previous
"""Trainium2 Bass kernel for a ViT attention block (LN -> QKV -> RoPE -> attn -> out-proj).

Sharding: 8 cores = 2 batches x 4 head-groups (4 heads each). Each core computes
a partial out-projection (its 4 heads) for one batch, transposed as [D, N].
Host sums the 4 partials per batch and transposes back. LayerNorm gamma/beta are
folded into the QKV weights/bias on the host.

Device layout (per core):
  - LN in [tok, d] layout (bn_stats per-partition), normalized output cast bf16,
    PE-transposed into xT tiles ([d-chunk, tok] layout, split into 5 token
    groups so the QKV projection can start before LayerNorm finishes).
  - Q,K projected directly into [feat, tok] (lhsT = weight chunks); V projected
    into [tok, feat] (lhsT = xT chunks). QKV bias enters as a K=1 matmul row.
  - RoPE in [feat, tok] with host-built cos/sin tables ([128] rows = 2 heads
    stacked; cls token and padding encoded in the tables). The rotate-half
    partner comes from partition-shifting SBUF->SBUF DMAs.
  - Attention transposed: dotsT[k,q] = matmul(lhsT=kT, rhs=qT), exp on ACT
    (logits are bounded, no max subtraction), attn@v as outT[65,q] with an
    appended ones column in V giving the softmax denominator for free.
  - Denominators are DMA'd into 32-aligned partitions of one tile so a single
    full-width reciprocal+cast serves all 4 heads of a q-chunk; normalize via
    ones outer-product broadcast (PE) + DVE multiply; out-proj accumulates the
    256 head dims; result DMA'd out as [D, N] (transposed).
All matmuls bf16 with f32 PSUM accumulation.
"""

import sys

sys.path.insert(0, "/opt/trn_rl_repo")

import numpy as np
import ml_dtypes

import concourse.bacc as bacc
import concourse.mybir as mybir
import concourse.tile as tile
from concourse.bass_utils import run_bass_kernel_spmd

F32 = mybir.dt.float32
BF16 = mybir.dt.bfloat16
AF = mybir.ActivationFunctionType
OP = mybir.AluOpType
BF = ml_dtypes.bfloat16

B, N, D = 2, 2049, 1024
DH = 64
HPC = 4  # heads per core
NT = 17  # 128-token tiles (padded to 2176)
TPAD = NT * 128
SCALE = DH ** -0.5
# q-column chunks [offset, width]; the tail chunk is the single real token 2048
JJ = [(2048, 1), (0, 1024), (1024, 1024)]
J5 = [(0, 512), (512, 512), (1024, 512), (1536, 512), (2048, 128)]
# token groups backing the 5 xT tiles (4+4+4+4+1 of the 17 token tiles)
TG = [(0, 512), (512, 512), (1024, 512), (1536, 512), (2048, 128)]


def _subs(jw):
    return [(s, min(512, jw - s)) for s in range(0, jw, 512)]


def _tg_of(col):
    return min(col // 512, 4)


def _build():
    nc = bacc.Bacc("TRN2", target_bir_lowering=False, debug=False, num_devices=8)

    x_d = nc.declare_dram_parameter("x", [N, D], F32, False)
    wqk_d = nc.declare_dram_parameter("wqk", [D, 512], BF16, False)
    wv_d = nc.declare_dram_parameter("wv", [D, 256], BF16, False)
    wo_d = nc.declare_dram_parameter("wo", [256, D], BF16, False)
    bqk_d = nc.declare_dram_parameter("bqk", [1, 512], BF16, False)
    bv_d = nc.declare_dram_parameter("bv", [1, 256], BF16, False)
    cos_d = nc.declare_dram_parameter("cos2", [128, TPAD], BF16, False)
    sin_d = nc.declare_dram_parameter("sinf2", [128, TPAD], BF16, False)
    idn_d = nc.declare_dram_parameter("ident", [128, 128], BF16, False)
    out_d = nc.declare_dram_parameter("out", [D, N], F32, True)

    with tile.TileContext(nc) as tc:
        with (
            tc.tile_pool(name="const", bufs=1) as cpool,
            tc.tile_pool(name="persist", bufs=1) as ppool,
            tc.tile_pool(name="work", bufs=2) as wpool,
            tc.tile_pool(name="psum", bufs=2, space="PSUM") as pspool,
        ):
            # ---------------- constants ----------------
            wqk_sb = [cpool.tile([128, 512], BF16, tag=f"wqk{c}", name=f"wqk{c}") for c in range(8)]
            wv_sb = [cpool.tile([128, 256], BF16, tag=f"wv{c}", name=f"wv{c}") for c in range(8)]
            wo_sb = [cpool.tile([128, 1024], BF16, tag=f"wo{c}", name=f"wo{c}") for c in range(2)]
            bqk_sb = cpool.tile([1, 512], BF16, tag="bqk", name="bqk")
            bv_sb = cpool.tile([1, 256], BF16, tag="bv", name="bv")
            cos_sb = cpool.tile([128, TPAD], BF16, tag="cos", name="cos")
            sin_sb = cpool.tile([128, TPAD], BF16, tag="sin", name="sin")
            idn_sb = cpool.tile([128, 128], BF16, tag="idn", name="idn")
            ones_sb = cpool.tile([1, TPAD], BF16, tag="ones", name="ones")

            nc.sync.dma_start(out=idn_sb[:], in_=idn_d[:])

            def _load_weights():
                for c in range(8):
                    nc.sync.dma_start(out=wqk_sb[c][:], in_=wqk_d[c * 128:(c + 1) * 128, :])
                    nc.sync.dma_start(out=wv_sb[c][:], in_=wv_d[c * 128:(c + 1) * 128, :])

            def _load_consts():
                for c in range(2):
                    nc.gpsimd.dma_start(out=wo_sb[c][:], in_=wo_d[c * 128:(c + 1) * 128, :])
                nc.gpsimd.dma_start(out=bqk_sb[:], in_=bqk_d[:])
                nc.gpsimd.dma_start(out=bv_sb[:], in_=bv_d[:])
                nc.gpsimd.dma_start(out=cos_sb[:], in_=cos_d[:])
                nc.gpsimd.dma_start(out=sin_sb[:], in_=sin_d[:])

            nc.vector.memset(ones_sb[:], 1.0)
            eps_sb = cpool.tile([128, 1], F32, tag="eps", name="eps")
            nc.vector.memset(eps_sb[:], 1e-5)
            ones64_sb = cpool.tile([128, 64], BF16, tag="ones64", name="ones64")
            nc.vector.memset(ones64_sb[:], 1.0)

            # ---------------- persistent activations ----------------
            # xT[g]: [128, 8 dchunks, tg-width] per token group g (fine-grained deps)
            xTg = [
                ppool.tile([128, 8 * tw], BF16, tag=f"xT{g}", name=f"xT{g}")
                for g, (to, tw) in enumerate(TG)
            ]
            xT3 = [
                xTg[g][:, :].rearrange("p (c t) -> p c t", c=8) for g in range(5)
            ]

            def xslice(c, jo, jw):
                g = _tg_of(jo)
                to, tw = TG[g]
                assert jo + jw <= to + tw
                return xT3[g][:, c, jo - to:jo - to + jw]

            # qkT tiles: 0,1 = q head-pairs (h01, h23); 2,3 = k head-pairs
            qkT_sb = [ppool.tile([128, TPAD], BF16, tag=f"qkT{f}", name=f"qkT{f}") for f in range(4)]
            vaug_sb = [ppool.tile([128, 260], BF16, tag=f"v{k}", name=f"v{k}") for k in range(NT)]

            # ---------------- phase A: LayerNorm + transpose ----------------
            for i in range(NT):
                xa = wpool.tile([128, D], F32, tag="xa", name="xa", bufs=3)
                if i < 16:
                    nc.sync.dma_start(out=xa[:], in_=x_d[i * 128:(i + 1) * 128, :])
                else:
                    nc.vector.memset(xa[:], 0.0)
                    nc.sync.dma_start(out=xa[0:1, :], in_=x_d[2048:2049, :])
                if i == 2:
                    _load_weights()
                if i == 4:
                    _load_consts()
                stats = wpool.tile([128, 12], F32, tag="stats", name="stats", bufs=3)
                mv = wpool.tile([128, 2], F32, tag="mv", name="mv", bufs=4)
                nc.vector.bn_stats(stats[:, 0:6], xa[:, 0:512])
                nc.vector.bn_stats(stats[:, 6:12], xa[:, 512:1024])
                nc.vector.bn_aggr(mv[:], stats[:])
                std = wpool.tile([128, 1], F32, tag="std", name="std", bufs=4)
                rstd = wpool.tile([128, 1], F32, tag="rstd", name="rstd", bufs=4)
                murstd = wpool.tile([128, 1], F32, tag="murstd", name="murstd")
                nc.scalar.activation(std[:], mv[:, 1:2], AF.Sqrt, bias=eps_sb[:])
                nc.vector.reciprocal(rstd[:], std[:])
                nc.vector.tensor_mul(murstd[:], mv[:, 0:1], rstd[:])
                xn = wpool.tile([128, D], BF16, tag="xn", name="xn", bufs=4)
                nc.vector.tensor_scalar(
                    xn[:], xa[:], rstd[:], murstd[:], OP.mult, OP.subtract
                )
                g = _tg_of(i * 128)
                to, tw = TG[g]
                for s in range(2):
                    pst = pspool.tile([128, 512], BF16, tag="dots", name="pst", bufs=3)
                    for c in range(4):
                        nc.tensor.transpose(
                            pst[:, c * 128:(c + 1) * 128],
                            xn[:, (4 * s + c) * 128:(4 * s + c + 1) * 128],
                            idn_sb[:],
                        )
                    nc.vector.tensor_copy(
                        xT3[g][:, 4 * s:4 * s + 4, i * 128 - to:(i + 1) * 128 - to],
                        pst[:, :].rearrange("p (c t) -> p c t", c=4),
                    )

            # ---------------- phase B1: Q,K projection + RoPE ----------------
            for f in (0, 2, 1, 3):
                qf = wpool.tile([128, TPAD], F32, tag="qf", name="qf", bufs=2)
                for jo, jw in J5:
                    psq = pspool.tile([128, 512], F32, tag="po", name="psq", bufs=2)
                    for c in range(8):
                        nc.tensor.matmul(
                            psq[:, 0:jw],
                            wqk_sb[c][:, f * 128:(f + 1) * 128],
                            xslice(c, jo, jw),
                            start=(c == 0),
                            stop=False,
                        )
                    nc.tensor.matmul(
                        psq[:, 0:jw],
                        bqk_sb[:, f * 128:(f + 1) * 128],
                        ones_sb[:, jo:jo + jw],
                        start=False,
                        stop=True,
                    )
                    nc.scalar.copy(qf[:, jo:jo + jw], psq[:, 0:jw])
                # rotate-half partner via partition-shifting DMAs
                qs = wpool.tile([128, TPAD], F32, tag="qs", name="qs", bufs=1)
                nc.sync.dma_start(out=qs[0:32, :], in_=qf[32:64, :])
                nc.sync.dma_start(out=qs[32:64, :], in_=qf[0:32, :])
                nc.sync.dma_start(out=qs[64:96, :], in_=qf[96:128, :])
                nc.sync.dma_start(out=qs[96:128, :], in_=qf[64:96, :])
                for jo, jw in J5:
                    t1 = wpool.tile([128, 512], F32, tag="t1", name="t1", bufs=2)
                    t2 = wpool.tile([128, 512], F32, tag="t2", name="t2", bufs=2)
                    nc.vector.tensor_mul(
                        t1[:, 0:jw], qf[:, jo:jo + jw], cos_sb[:, jo:jo + jw]
                    )
                    nc.vector.tensor_mul(
                        t2[:, 0:jw], qs[:, jo:jo + jw], sin_sb[:, jo:jo + jw]
                    )
                    nc.vector.tensor_add(
                        qkT_sb[f][:, jo:jo + jw], t1[:, 0:jw], t2[:, 0:jw]
                    )

            # ---------------- phase B2: V projection -> v_aug ----------------
            for k in range(NT):
                psv = pspool.tile([128, 256], F32, tag="po", name="psv", bufs=2)
                for c in range(8):
                    nc.tensor.matmul(
                        psv[:],
                        xslice(c, k * 128, 128),
                        wv_sb[c][:],
                        start=(c == 0),
                        stop=False,
                    )
                nc.tensor.matmul(
                    psv[:],
                    ones_sb[:, k * 128:(k + 1) * 128],
                    bv_sb[:],
                    start=False,
                    stop=True,
                )
                va = vaug_sb[k]
                va3 = va[:, :].rearrange("p (a b) -> p a b", a=4)
                if k < 16:
                    nc.scalar.copy(
                        va3[:, :, 0:64], psv[:, :].rearrange("p (a b) -> p a b", a=4)
                    )
                    nc.vector.memset(va3[:, :, 64:65], 1.0)
                else:
                    # only token 2048 is real; zero rows kill padded keys
                    nc.vector.memset(va[:], 0.0)
                    va3r = va[0:1, :].rearrange("p (a b) -> p a b", a=4)
                    nc.scalar.copy(
                        va3r[:, :, 0:64],
                        psv[0:1, :].rearrange("p (a b) -> p a b", a=4),
                    )
                    nc.vector.memset(va3r[:, :, 64:65], 1.0)

            # ---------------- phases C+D: attention, normalize, out-proj ----------------
            # The epilogue (normalize + out-proj) of chunk j is emitted AFTER
            # chunk j+1's attention stream: the PE queue is in-order, so an
            # epilogue waiting on the reciprocal chain would otherwise block
            # the next chunk's dots and starve ACT.

            def attention(jo, jw):
                oev_l = []
                dnp = [
                    wpool.tile([64, 1024], BF16, tag=f"dnp{p}", name=f"dnp{p}", bufs=2)
                    for p in range(2)
                ]
                for h in range(HPC):
                    qt = qkT_sb[h // 2]
                    ktile = qkT_sb[2 + h // 2]
                    po = (h % 2) * 64
                    pso = [
                        pspool.tile([65, 512], F32, tag="po", name=f"pso{si}", bufs=2)
                        for si in range(2 if jw > 1 else 1)
                    ]
                    if jw == 1:
                        psd = pspool.tile([128, NT], F32, tag="dots", name="psdt", bufs=3)
                        for k in range(NT):
                            nc.tensor.matmul(
                                psd[:, k:k + 1],
                                ktile[po:po + 64, k * 128:(k + 1) * 128],
                                qt[po:po + 64, jo:jo + 1],
                            )
                        ex = wpool.tile([128, 1024], BF16, tag="ex", name="ex", bufs=4)
                        nc.scalar.activation(ex[:, 0:NT], psd[:], AF.Exp, scale=SCALE)
                        for k in range(NT):
                            nc.tensor.matmul(
                                pso[0][:, 0:1],
                                vaug_sb[k][:, h * 65:h * 65 + 65],
                                ex[:, k:k + 1],
                                start=(k == 0),
                                stop=(k == NT - 1),
                                skip_group_check=True,
                            )
                    else:
                        for k in range(NT):
                            psd = pspool.tile([128, 1024], F32, tag="dots", name="psd", bufs=3)
                            for so, sw in _subs(jw):
                                nc.tensor.matmul(
                                    psd[:, so:so + sw],
                                    ktile[po:po + 64, k * 128:(k + 1) * 128],
                                    qt[po:po + 64, jo + so:jo + so + sw],
                                )
                            ex = wpool.tile([128, 1024], BF16, tag="ex", name="ex", bufs=4)
                            nc.scalar.activation(
                                ex[:, 0:jw], psd[:, 0:jw], AF.Exp, scale=SCALE
                            )
                            for si, (so, sw) in enumerate(_subs(jw)):
                                nc.tensor.matmul(
                                    pso[si][:, 0:sw],
                                    vaug_sb[k][:, h * 65:h * 65 + 65],
                                    ex[:, so:so + sw],
                                    start=(k == 0),
                                    stop=(k == NT - 1),
                                    skip_group_check=True,
                                )
                    # evacuate quickly to free the PSUM slots
                    oev = (wpool.tile([65, 64], BF16, tag="oevt", name="oevt", bufs=4)
                           if jw == 1 else
                           wpool.tile([65, 1024], BF16, tag="oev", name="oev", bufs=4))
                    for si, (so, sw) in enumerate(_subs(jw)):
                        nc.vector.tensor_copy(oev[:, so:so + sw], pso[si][:, 0:sw])
                    oev_l.append(oev)
                    # denominator row -> 32-aligned partition of the packed tile
                    nc.gpsimd.dma_start(
                        out=dnp[h // 2][32 * (h % 2):32 * (h % 2) + 1, 0:jw],
                        in_=oev[64:65, 0:jw],
                    )
                return oev_l, dnp

            def epilogue(jo, jw, oev_l, dnp):
                an = [
                    wpool.tile([128, 1024], BF16, tag=f"an{ch}", name=f"an{ch}", bufs=2)
                    for ch in range(2)
                ]
                dnb = [
                    wpool.tile([64, 1024], BF16, tag=f"dnb{p}", name=f"dnb{p}", bufs=2)
                    for p in range(2)
                ]
                for p in range(2):
                    for so, sw in _subs(jw):
                        dnr = wpool.tile([64, 512], F32, tag="dnr", name="dnr", bufs=2)
                        nc.vector.reciprocal(dnr[:, 0:sw], dnp[p][:, so:so + sw])
                        nc.vector.tensor_copy(dnb[p][:, so:so + sw], dnr[:, 0:sw])
                for h in range(HPC):
                    for so, sw in _subs(jw):
                        psb = pspool.tile([64, 512], F32, tag="po", name="psb", bufs=2)
                        hp, hr = h // 2, 32 * (h % 2)
                        nc.tensor.matmul(
                            psb[:, 0:sw],
                            ones64_sb[hr:hr + 1, :],
                            dnb[hp][hr:hr + 1, so:so + sw],
                        )
                        nt = wpool.tile([64, 512], BF16, tag="nt", name="nt", bufs=4)
                        nc.vector.tensor_mul(
                            nt[:, 0:sw], oev_l[h][0:64, so:so + sw], psb[:, 0:sw]
                        )
                        nc.gpsimd.dma_start(
                            out=an[h // 2][(h % 2) * 64:(h % 2) * 64 + 64, so:so + sw],
                            in_=nt[:, 0:sw],
                        )
                for of in range(8):
                    for so, sw in _subs(jw):
                        swr = min(sw, max(0, N - (jo + so)))
                        pp = pspool.tile([128, 512], F32, tag="po", name="pp", bufs=2)
                        nc.tensor.matmul(
                            pp[:, 0:sw],
                            wo_sb[0][:, of * 128:(of + 1) * 128],
                            an[0][:, so:so + sw],
                            start=True,
                            stop=False,
                        )
                        nc.tensor.matmul(
                            pp[:, 0:sw],
                            wo_sb[1][:, of * 128:(of + 1) * 128],
                            an[1][:, so:so + sw],
                            start=False,
                            stop=True,
                        )
                        oo = wpool.tile([128, 512], F32, tag="oo", name="oo", bufs=2)
                        nc.vector.tensor_copy(oo[:, 0:swr], pp[:, 0:swr])
                        nc.sync.dma_start(
                            out=out_d[of * 128:(of + 1) * 128, jo + so:jo + so + swr],
                            in_=oo[:, 0:swr],
                        )

            for jo, jw in JJ:
                oev_l, dnp = attention(jo, jw)
                epilogue(jo, jw, oev_l, dnp)
    nc.compile()
    return nc


_NC = None


def _get_nc():
    global _NC
    if _NC is None:
        _NC = _build()
    return _NC


def _make_inputs(x, ln_gamma, ln_beta, w_qkv, w_out):
    w_eff = (w_qkv * ln_gamma[:, None].astype(np.float32)).astype(np.float32)
    b_all = (ln_beta.astype(np.float32) @ w_qkv.astype(np.float32)).astype(np.float32)

    inv = 1.0 / (10000.0 ** (np.arange(0, 64, 2, dtype=np.float64) / 64.0))
    fr = np.arange(2048, dtype=np.float64)[:, None] * inv[None, :]
    cos64 = np.concatenate([np.cos(fr), np.cos(fr)], axis=1).T  # [64, 2048]
    sinf64 = np.concatenate([-np.sin(fr), np.sin(fr)], axis=1).T
    cos2 = np.ones((128, TPAD), np.float32)
    sinf2 = np.zeros((128, TPAD), np.float32)
    cos2[:, 1:2049] = np.tile(cos64, (2, 1)).astype(np.float32)
    sinf2[:, 1:2049] = np.tile(sinf64, (2, 1)).astype(np.float32)
    ident = np.eye(128, dtype=BF)

    in_maps = []
    for c in range(8):
        b, g = c // 4, c % 4
        cols = slice(256 * g, 256 * g + 256)
        wqk = np.concatenate(
            [w_eff[:, 0:1024][:, cols], w_eff[:, 1024:2048][:, cols]], axis=1
        ).astype(BF)
        wv = w_eff[:, 2048:3072][:, cols].astype(BF)
        wo = w_out[cols, :].astype(BF)
        bqk = np.concatenate([b_all[0:1024][cols], b_all[1024:2048][cols]])[None, :].astype(BF)
        bv = b_all[2048:3072][cols][None, :].astype(BF)
        in_maps.append(
            {
                "x": np.ascontiguousarray(x[b]).astype(np.float32),
                "wqk": np.ascontiguousarray(wqk),
                "wv": np.ascontiguousarray(wv),
                "wo": np.ascontiguousarray(wo),
                "bqk": bqk,
                "bv": bv,
                "cos2": cos2.astype(BF),
                "sinf2": sinf2.astype(BF),
                "ident": ident,
            }
        )
    return in_maps


def kernel(x, ln_gamma, ln_beta, w_qkv, w_out, _trace=False, _trace_kwargs=None):
    nc = _get_nc()
    in_maps = _make_inputs(x, ln_gamma, ln_beta, w_qkv, w_out)
    res = run_bass_kernel_spmd(
        nc, in_maps, core_ids=list(range(8)), trace=_trace,
        **(_trace_kwargs or {}),
    )
    out = np.zeros((B, N, D), np.float32)
    for c in range(8):
        out[c // 4] += np.asarray(res.results[c]["out"], np.float32).T
    if _trace:
        return out, res
    return out


# revision 38
# speedup vs baseline: 1.1733x; 1.0136x over previous
"""Trainium2 Bass kernel for a ViT attention block (LN -> QKV -> RoPE -> attn -> out-proj).

Sharding: 8 cores = 2 batches x 4 head-groups (4 heads each). Each core computes
a partial out-projection (its 4 heads) for one batch, transposed as [D, N].
Host sums the 4 partials per batch and transposes back. LayerNorm gamma/beta are
folded into the QKV weights/bias on the host.

Device layout (per core):
  - LN in [tok, d] layout (bn_stats per-partition), normalized output cast bf16,
    PE-transposed into xT tiles ([d-chunk, tok] layout, split into 5 token
    groups so the QKV projection can start before LayerNorm finishes).
  - Q,K projected directly into [feat, tok] (lhsT = weight chunks); V projected
    into [tok, feat] (lhsT = xT chunks). QKV bias enters as a K=1 matmul row.
  - RoPE in [feat, tok] with host-built cos/sin tables ([128] rows = 2 heads
    stacked; cls token and padding encoded in the tables). The rotate-half
    partner comes from partition-shifting SBUF->SBUF DMAs.
  - Attention transposed: dotsT[k,q] = matmul(lhsT=kT, rhs=qT), exp on ACT
    (logits are bounded, no max subtraction), attn@v as outT[65,q] with an
    appended ones column in V giving the softmax denominator for free.
  - Denominators are DMA'd into 32-aligned partitions of one tile so a single
    full-width reciprocal+cast serves all 4 heads of a q-chunk; normalize via
    ones outer-product broadcast (PE) + DVE multiply; out-proj accumulates the
    256 head dims; result DMA'd out as [D, N] (transposed).
All matmuls bf16 with f32 PSUM accumulation.
"""

import sys

sys.path.insert(0, "/opt/trn_rl_repo")

import numpy as np
import ml_dtypes

import concourse.bacc as bacc
import concourse.mybir as mybir
import concourse.tile as tile
from concourse.bass_utils import run_bass_kernel_spmd

F32 = mybir.dt.float32
BF16 = mybir.dt.bfloat16
AF = mybir.ActivationFunctionType
OP = mybir.AluOpType
BF = ml_dtypes.bfloat16

B, N, D = 2, 2049, 1024
DH = 64
HPC = 4  # heads per core
NT = 17  # 128-token tiles (padded to 2176)
TPAD = NT * 128
SCALE = DH ** -0.5
# q-column chunks [offset, width]; the tail chunk is the single real token 2048
JJ = [(2048, 1), (0, 1024), (1024, 1024)]
J5 = [(0, 512), (512, 512), (1024, 512), (1536, 512), (2048, 128)]
# token groups backing the 5 xT tiles (4+4+4+4+1 of the 17 token tiles)
TG = [(0, 512), (512, 512), (1024, 512), (1536, 512), (2048, 128)]


def _subs(jw):
    return [(s, min(512, jw - s)) for s in range(0, jw, 512)]


def _tg_of(col):
    return min(col // 512, 4)


def _build():
    nc = bacc.Bacc("TRN2", target_bir_lowering=False, debug=False, num_devices=8)

    x_d = nc.declare_dram_parameter("x", [N, D], F32, False)
    wqk_d = nc.declare_dram_parameter("wqk", [D, 512], BF16, False)
    wv_d = nc.declare_dram_parameter("wv", [D, 256], BF16, False)
    wo_d = nc.declare_dram_parameter("wo", [256, D], BF16, False)
    bqk_d = nc.declare_dram_parameter("bqk", [1, 512], BF16, False)
    bv_d = nc.declare_dram_parameter("bv", [1, 256], BF16, False)
    cos_d = nc.declare_dram_parameter("cos2", [128, TPAD], BF16, False)
    sin_d = nc.declare_dram_parameter("sinf2", [128, TPAD], BF16, False)
    idn_d = nc.declare_dram_parameter("ident", [128, 128], BF16, False)
    out_d = nc.declare_dram_parameter("out", [D, N], F32, True)

    with tile.TileContext(nc) as tc:
        with (
            tc.tile_pool(name="const", bufs=1) as cpool,
            tc.tile_pool(name="persist", bufs=1) as ppool,
            tc.tile_pool(name="work", bufs=2) as wpool,
            tc.tile_pool(name="psum", bufs=2, space="PSUM") as pspool,
        ):
            # ---------------- constants ----------------
            wqk_sb = [cpool.tile([128, 512], BF16, tag=f"wqk{c}", name=f"wqk{c}") for c in range(8)]
            wv_sb = [cpool.tile([128, 256], BF16, tag=f"wv{c}", name=f"wv{c}") for c in range(8)]
            wo_sb = [cpool.tile([128, 1024], BF16, tag=f"wo{c}", name=f"wo{c}") for c in range(2)]
            bqk_sb = cpool.tile([1, 512], BF16, tag="bqk", name="bqk")
            bv_sb = cpool.tile([1, 256], BF16, tag="bv", name="bv")
            cos_sb = cpool.tile([128, TPAD], BF16, tag="cos", name="cos")
            sin_sb = cpool.tile([128, TPAD], BF16, tag="sin", name="sin")
            idn_sb = cpool.tile([128, 128], BF16, tag="idn", name="idn")
            ones_sb = cpool.tile([1, TPAD], BF16, tag="ones", name="ones")

            nc.sync.dma_start(out=idn_sb[:], in_=idn_d[:])

            def _load_weights():
                for c in range(8):
                    nc.sync.dma_start(out=wqk_sb[c][:], in_=wqk_d[c * 128:(c + 1) * 128, :])
                    nc.sync.dma_start(out=wv_sb[c][:], in_=wv_d[c * 128:(c + 1) * 128, :])

            def _load_consts():
                for c in range(2):
                    nc.gpsimd.dma_start(out=wo_sb[c][:], in_=wo_d[c * 128:(c + 1) * 128, :])
                nc.gpsimd.dma_start(out=bqk_sb[:], in_=bqk_d[:])
                nc.gpsimd.dma_start(out=bv_sb[:], in_=bv_d[:])
                nc.gpsimd.dma_start(out=cos_sb[:], in_=cos_d[:])
                nc.gpsimd.dma_start(out=sin_sb[:], in_=sin_d[:])

            nc.vector.memset(ones_sb[:], 1.0)
            eps_sb = cpool.tile([128, 1], F32, tag="eps", name="eps")
            nc.vector.memset(eps_sb[:], 1e-5)
            ones64_sb = cpool.tile([128, 64], BF16, tag="ones64", name="ones64")
            nc.vector.memset(ones64_sb[:], 1.0)

            # ---------------- persistent activations ----------------
            # xT[g]: [128, 8 dchunks, tg-width] per token group g (fine-grained deps)
            xTg = [
                ppool.tile([128, 8 * tw], BF16, tag=f"xT{g}", name=f"xT{g}")
                for g, (to, tw) in enumerate(TG)
            ]
            xT3 = [
                xTg[g][:, :].rearrange("p (c t) -> p c t", c=8) for g in range(5)
            ]

            def xslice(c, jo, jw):
                g = _tg_of(jo)
                to, tw = TG[g]
                assert jo + jw <= to + tw
                return xT3[g][:, c, jo - to:jo - to + jw]

            # qkT tiles: 0,1 = q head-pairs (h01, h23); 2,3 = k head-pairs
            qkT_sb = [ppool.tile([128, TPAD], BF16, tag=f"qkT{f}", name=f"qkT{f}") for f in range(4)]
            vaug_sb = [ppool.tile([128, 260], BF16, tag=f"v{k}", name=f"v{k}") for k in range(NT)]

            # ---------------- phase A: LayerNorm + transpose ----------------
            for i in range(NT):
                xa = wpool.tile([128, D], F32, tag="xa", name="xa", bufs=3)
                if i < 16:
                    nc.sync.dma_start(out=xa[:], in_=x_d[i * 128:(i + 1) * 128, :])
                else:
                    nc.vector.memset(xa[:], 0.0)
                    nc.sync.dma_start(out=xa[0:1, :], in_=x_d[2048:2049, :])
                if i == 2:
                    _load_weights()
                if i == 4:
                    _load_consts()
                stats = wpool.tile([128, 12], F32, tag="stats", name="stats", bufs=3)
                mv = wpool.tile([128, 2], F32, tag="mv", name="mv", bufs=4)
                nc.vector.bn_stats(stats[:, 0:6], xa[:, 0:512])
                nc.vector.bn_stats(stats[:, 6:12], xa[:, 512:1024])
                nc.vector.bn_aggr(mv[:], stats[:])
                std = wpool.tile([128, 1], F32, tag="std", name="std", bufs=4)
                rstd = wpool.tile([128, 1], F32, tag="rstd", name="rstd", bufs=4)
                murstd = wpool.tile([128, 1], F32, tag="murstd", name="murstd")
                nc.scalar.activation(std[:], mv[:, 1:2], AF.Sqrt, bias=eps_sb[:])
                nc.vector.reciprocal(rstd[:], std[:])
                nc.vector.tensor_mul(murstd[:], mv[:, 0:1], rstd[:])
                xn = wpool.tile([128, D], BF16, tag="xn", name="xn", bufs=4)
                nc.vector.tensor_scalar(
                    xn[:], xa[:], rstd[:], murstd[:], OP.mult, OP.subtract
                )
                g = _tg_of(i * 128)
                to, tw = TG[g]
                for s in range(2):
                    pst = pspool.tile([128, 512], BF16, tag="dots", name="pst", bufs=3)
                    for c in range(4):
                        nc.tensor.transpose(
                            pst[:, c * 128:(c + 1) * 128],
                            xn[:, (4 * s + c) * 128:(4 * s + c + 1) * 128],
                            idn_sb[:],
                        )
                    nc.vector.tensor_copy(
                        xT3[g][:, 4 * s:4 * s + 4, i * 128 - to:(i + 1) * 128 - to],
                        pst[:, :].rearrange("p (c t) -> p c t", c=4),
                    )

            # ---------------- phase B1: Q,K projection + RoPE ----------------
            for f in (0, 2, 1, 3):
                qf = wpool.tile([128, TPAD], F32, tag="qf", name="qf", bufs=2)
                for jo, jw in J5:
                    psq = pspool.tile([128, 512], F32, tag="po", name="psq", bufs=2)
                    for c in range(8):
                        nc.tensor.matmul(
                            psq[:, 0:jw],
                            wqk_sb[c][:, f * 128:(f + 1) * 128],
                            xslice(c, jo, jw),
                            start=(c == 0),
                            stop=False,
                        )
                    nc.tensor.matmul(
                        psq[:, 0:jw],
                        bqk_sb[:, f * 128:(f + 1) * 128],
                        ones_sb[:, jo:jo + jw],
                        start=False,
                        stop=True,
                    )
                    nc.scalar.copy(qf[:, jo:jo + jw], psq[:, 0:jw])
                # rotate-half partner via partition-shifting DMAs
                qs = wpool.tile([128, TPAD], F32, tag="qs", name="qs", bufs=1)
                nc.sync.dma_start(out=qs[0:32, :], in_=qf[32:64, :])
                nc.sync.dma_start(out=qs[32:64, :], in_=qf[0:32, :])
                nc.sync.dma_start(out=qs[64:96, :], in_=qf[96:128, :])
                nc.sync.dma_start(out=qs[96:128, :], in_=qf[64:96, :])
                for jo, jw in J5:
                    t1 = wpool.tile([128, 512], F32, tag="t1", name="t1", bufs=2)
                    t2 = wpool.tile([128, 512], F32, tag="t2", name="t2", bufs=2)
                    nc.vector.tensor_mul(
                        t1[:, 0:jw], qf[:, jo:jo + jw], cos_sb[:, jo:jo + jw]
                    )
                    nc.vector.tensor_mul(
                        t2[:, 0:jw], qs[:, jo:jo + jw], sin_sb[:, jo:jo + jw]
                    )
                    nc.vector.tensor_add(
                        qkT_sb[f][:, jo:jo + jw], t1[:, 0:jw], t2[:, 0:jw]
                    )

            # ---------------- phase B2: V projection -> v_aug ----------------
            for k in range(NT):
                psv = pspool.tile([128, 256], F32, tag="po", name="psv", bufs=2)
                for c in range(8):
                    nc.tensor.matmul(
                        psv[:],
                        xslice(c, k * 128, 128),
                        wv_sb[c][:],
                        start=(c == 0),
                        stop=False,
                    )
                nc.tensor.matmul(
                    psv[:],
                    ones_sb[:, k * 128:(k + 1) * 128],
                    bv_sb[:],
                    start=False,
                    stop=True,
                )
                va = vaug_sb[k]
                va3 = va[:, :].rearrange("p (a b) -> p a b", a=4)
                if k < 16:
                    nc.scalar.copy(
                        va3[:, :, 0:64], psv[:, :].rearrange("p (a b) -> p a b", a=4)
                    )
                    nc.vector.memset(va3[:, :, 64:65], 1.0)
                else:
                    # only token 2048 is real; zero rows kill padded keys
                    nc.vector.memset(va[:], 0.0)
                    va3r = va[0:1, :].rearrange("p (a b) -> p a b", a=4)
                    nc.scalar.copy(
                        va3r[:, :, 0:64],
                        psv[0:1, :].rearrange("p (a b) -> p a b", a=4),
                    )
                    nc.vector.memset(va3r[:, :, 64:65], 1.0)

            # ---------------- phases C+D: attention, normalize, out-proj ----------------
            # Heads are emitted h0,h1,h2,[pair0 epilogue],h3,[pair1 epilogue],
            # out-proj: pair0's reciprocal chain resolves during h2's attention
            # so the in-order PE queue never stalls on it; even heads write
            # their normalized tile straight into an[] on DVE (partition 0:64).

            def att_head(jo, jw, h, oev_l, dnp):
                qt = qkT_sb[h // 2]
                ktile = qkT_sb[2 + h // 2]
                po = (h % 2) * 64
                pso = [
                    pspool.tile([65, 512], F32, tag="po", name=f"pso{si}", bufs=2)
                    for si in range(2 if jw > 1 else 1)
                ]
                if jw == 1:
                    psd = pspool.tile([128, NT], F32, tag="dots", name="psdt", bufs=3)
                    for k in range(NT):
                        nc.tensor.matmul(
                            psd[:, k:k + 1],
                            ktile[po:po + 64, k * 128:(k + 1) * 128],
                            qt[po:po + 64, jo:jo + 1],
                        )
                    ex = wpool.tile([128, 1024], BF16, tag="ex", name="ex", bufs=4)
                    nc.scalar.activation(ex[:, 0:NT], psd[:], AF.Exp, scale=SCALE)
                    for k in range(NT):
                        nc.tensor.matmul(
                            pso[0][:, 0:1],
                            vaug_sb[k][:, h * 65:h * 65 + 65],
                            ex[:, k:k + 1],
                            start=(k == 0),
                            stop=(k == NT - 1),
                            skip_group_check=True,
                        )
                else:
                    for k in range(NT):
                        psd = pspool.tile([128, 1024], F32, tag="dots", name="psd", bufs=3)
                        for so, sw in _subs(jw):
                            nc.tensor.matmul(
                                psd[:, so:so + sw],
                                ktile[po:po + 64, k * 128:(k + 1) * 128],
                                qt[po:po + 64, jo + so:jo + so + sw],
                            )
                        ex = wpool.tile([128, 1024], BF16, tag="ex", name="ex", bufs=4)
                        nc.scalar.activation(
                            ex[:, 0:jw], psd[:, 0:jw], AF.Exp, scale=SCALE
                        )
                        for si, (so, sw) in enumerate(_subs(jw)):
                            nc.tensor.matmul(
                                pso[si][:, 0:sw],
                                vaug_sb[k][:, h * 65:h * 65 + 65],
                                ex[:, so:so + sw],
                                start=(k == 0),
                                stop=(k == NT - 1),
                                skip_group_check=True,
                            )
                oev = (wpool.tile([65, 64], BF16, tag="oevt", name="oevt", bufs=4)
                       if jw == 1 else
                       wpool.tile([65, 1024], BF16, tag="oev", name="oev", bufs=4))
                for si, (so, sw) in enumerate(_subs(jw)):
                    nc.vector.tensor_copy(oev[:, so:so + sw], pso[si][:, 0:sw])
                oev_l.append(oev)
                nc.gpsimd.dma_start(
                    out=dnp[h // 2][32 * (h % 2):32 * (h % 2) + 1, 0:jw],
                    in_=oev[64:65, 0:jw],
                )

            def epi_pair(jo, jw, p, oev_l, dnp, an):
                dnb = wpool.tile([64, 1024], BF16, tag=f"dnb{p}", name=f"dnb{p}", bufs=2)
                for so, sw in _subs(jw):
                    dnr = wpool.tile([64, 512], F32, tag="dnr", name="dnr", bufs=2)
                    nc.vector.reciprocal(dnr[:, 0:sw], dnp[p][:, so:so + sw])
                    nc.vector.tensor_copy(dnb[:, so:so + sw], dnr[:, 0:sw])
                for h in (2 * p, 2 * p + 1):
                    hr = 32 * (h % 2)
                    for so, sw in _subs(jw):
                        psb = pspool.tile([64, 512], F32, tag="po", name="psb", bufs=2)
                        nc.tensor.matmul(
                            psb[:, 0:sw],
                            ones64_sb[hr:hr + 1, :],
                            dnb[hr:hr + 1, so:so + sw],
                        )
                        if h % 2 == 0:
                            nc.vector.tensor_mul(
                                an[h // 2][0:64, so:so + sw],
                                oev_l[h][0:64, so:so + sw],
                                psb[:, 0:sw],
                            )
                        else:
                            nt = wpool.tile([64, 512], BF16, tag="nt", name="nt", bufs=4)
                            nc.vector.tensor_mul(
                                nt[:, 0:sw], oev_l[h][0:64, so:so + sw], psb[:, 0:sw]
                            )
                            nc.gpsimd.dma_start(
                                out=an[h // 2][64:128, so:so + sw], in_=nt[:, 0:sw]
                            )

            def outproj(jo, jw, an):
                for of in range(8):
                    for so, sw in _subs(jw):
                        swr = min(sw, max(0, N - (jo + so)))
                        pp = pspool.tile([128, 512], F32, tag="po", name="pp", bufs=2)
                        nc.tensor.matmul(
                            pp[:, 0:sw],
                            wo_sb[0][:, of * 128:(of + 1) * 128],
                            an[0][:, so:so + sw],
                            start=True,
                            stop=False,
                        )
                        nc.tensor.matmul(
                            pp[:, 0:sw],
                            wo_sb[1][:, of * 128:(of + 1) * 128],
                            an[1][:, so:so + sw],
                            start=False,
                            stop=True,
                        )
                        oo = wpool.tile([128, 512], F32, tag="oo", name="oo", bufs=2)
                        nc.vector.tensor_copy(oo[:, 0:swr], pp[:, 0:swr])
                        nc.sync.dma_start(
                            out=out_d[of * 128:(of + 1) * 128, jo + so:jo + so + swr],
                            in_=oo[:, 0:swr],
                        )

            for jo, jw in JJ:
                an = [
                    wpool.tile([128, 1024], BF16, tag=f"an{ch}", name=f"an{ch}", bufs=2)
                    for ch in range(2)
                ]
                dnp = [
                    wpool.tile([64, 1024], BF16, tag=f"dnp{p}", name=f"dnp{p}", bufs=2)
                    for p in range(2)
                ]
                oev_l = []
                att_head(jo, jw, 0, oev_l, dnp)
                att_head(jo, jw, 1, oev_l, dnp)
                att_head(jo, jw, 2, oev_l, dnp)
                epi_pair(jo, jw, 0, oev_l, dnp, an)
                att_head(jo, jw, 3, oev_l, dnp)
                epi_pair(jo, jw, 1, oev_l, dnp, an)
                outproj(jo, jw, an)
    nc.compile()
    return nc


_NC = None


def _get_nc():
    global _NC
    if _NC is None:
        _NC = _build()
    return _NC


def _make_inputs(x, ln_gamma, ln_beta, w_qkv, w_out):
    w_eff = (w_qkv * ln_gamma[:, None].astype(np.float32)).astype(np.float32)
    b_all = (ln_beta.astype(np.float32) @ w_qkv.astype(np.float32)).astype(np.float32)

    inv = 1.0 / (10000.0 ** (np.arange(0, 64, 2, dtype=np.float64) / 64.0))
    fr = np.arange(2048, dtype=np.float64)[:, None] * inv[None, :]
    cos64 = np.concatenate([np.cos(fr), np.cos(fr)], axis=1).T  # [64, 2048]
    sinf64 = np.concatenate([-np.sin(fr), np.sin(fr)], axis=1).T
    cos2 = np.ones((128, TPAD), np.float32)
    sinf2 = np.zeros((128, TPAD), np.float32)
    cos2[:, 1:2049] = np.tile(cos64, (2, 1)).astype(np.float32)
    sinf2[:, 1:2049] = np.tile(sinf64, (2, 1)).astype(np.float32)
    ident = np.eye(128, dtype=BF)

    in_maps = []
    for c in range(8):
        b, g = c // 4, c % 4
        cols = slice(256 * g, 256 * g + 256)
        wqk = np.concatenate(
            [w_eff[:, 0:1024][:, cols], w_eff[:, 1024:2048][:, cols]], axis=1
        ).astype(BF)
        wv = w_eff[:, 2048:3072][:, cols].astype(BF)
        wo = w_out[cols, :].astype(BF)
        bqk = np.concatenate([b_all[0:1024][cols], b_all[1024:2048][cols]])[None, :].astype(BF)
        bv = b_all[2048:3072][cols][None, :].astype(BF)
        in_maps.append(
            {
                "x": np.ascontiguousarray(x[b]).astype(np.float32),
                "wqk": np.ascontiguousarray(wqk),
                "wv": np.ascontiguousarray(wv),
                "wo": np.ascontiguousarray(wo),
                "bqk": bqk,
                "bv": bv,
                "cos2": cos2.astype(BF),
                "sinf2": sinf2.astype(BF),
                "ident": ident,
            }
        )
    return in_maps


def kernel(x, ln_gamma, ln_beta, w_qkv, w_out, _trace=False, _trace_kwargs=None):
    nc = _get_nc()
    in_maps = _make_inputs(x, ln_gamma, ln_beta, w_qkv, w_out)
    res = run_bass_kernel_spmd(
        nc, in_maps, core_ids=list(range(8)), trace=_trace,
        **(_trace_kwargs or {}),
    )
    out = np.zeros((B, N, D), np.float32)
    for c in range(8):
        out[c // 4] += np.asarray(res.results[c]["out"], np.float32).T
    if _trace:
        return out, res
    return out


# revision 39
# speedup vs baseline: 1.2522x; 1.0672x over previous
"""Trainium2 Bass kernel for a ViT attention block (LN -> QKV -> RoPE -> attn -> out-proj).

Sharding: 8 cores = 2 batches x 4 head-groups (4 heads each). Each core computes
a partial out-projection (its 4 heads) for one batch, transposed as [D, N].
Host sums the 4 partials per batch and transposes back. LayerNorm gamma/beta are
folded into the QKV weights/bias on the host.

Device layout (per core):
  - LN in [tok, d] layout (bn_stats per-partition), normalized output cast bf16,
    PE-transposed into xT tiles ([d-chunk, tok] layout, split into 5 token
    groups so the QKV projection can start before LayerNorm finishes).
  - Q,K projected directly into [feat, tok] (lhsT = weight chunks); V projected
    into [tok, feat] (lhsT = xT chunks). QKV bias enters as a K=1 matmul row.
  - RoPE in [feat, tok] with host-built cos/sin tables ([128] rows = 2 heads
    stacked; cls token and padding encoded in the tables). The rotate-half
    partner comes from partition-shifting SBUF->SBUF DMAs.
  - Attention transposed: dotsT[k,q] = matmul(lhsT=kT, rhs=qT), exp on ACT
    (logits are bounded, no max subtraction), attn@v as outT[65,q] with an
    appended ones column in V giving the softmax denominator for free.
  - Denominators are DMA'd into 32-aligned partitions of one tile so a single
    full-width reciprocal+cast serves all 4 heads of a q-chunk; normalize via
    ones outer-product broadcast (PE) + DVE multiply; out-proj accumulates the
    256 head dims; result DMA'd out as [D, N] (transposed).
All matmuls bf16 with f32 PSUM accumulation.
"""

import sys

sys.path.insert(0, "/opt/trn_rl_repo")

import numpy as np
import ml_dtypes

import concourse.bacc as bacc
import concourse.mybir as mybir
import concourse.tile as tile
from concourse.bass_utils import run_bass_kernel_spmd

F32 = mybir.dt.float32
BF16 = mybir.dt.bfloat16
AF = mybir.ActivationFunctionType
OP = mybir.AluOpType
BF = ml_dtypes.bfloat16

B, N, D = 2, 2049, 1024
DH = 64
HPC = 4  # heads per core
NT = 17  # 128-token tiles (padded to 2176)
TPAD = NT * 128
SCALE = DH ** -0.5
# q-column chunks [offset, width]; the tail chunk is the single real token 2048
JJ = [(2048, 1), (0, 1024), (1024, 1024)]
J5 = [(0, 512), (512, 512), (1024, 512), (1536, 512), (2048, 128)]
# token groups backing the 5 xT tiles (4+4+4+4+1 of the 17 token tiles)
TG = [(0, 512), (512, 512), (1024, 512), (1536, 512), (2048, 128)]


def _subs(jw):
    return [(s, min(512, jw - s)) for s in range(0, jw, 512)]


def _tg_of(col):
    return min(col // 512, 4)


def _build():
    nc = bacc.Bacc("TRN2", target_bir_lowering=False, debug=False, num_devices=8)

    x_d = nc.declare_dram_parameter("x", [N, D], F32, False)
    wqk_d = nc.declare_dram_parameter("wqk", [D, 512], BF16, False)
    wv_d = nc.declare_dram_parameter("wv", [D, 256], BF16, False)
    wo_d = nc.declare_dram_parameter("wo", [256, D], BF16, False)
    bqk_d = nc.declare_dram_parameter("bqk", [1, 512], BF16, False)
    bv_d = nc.declare_dram_parameter("bv", [1, 256], BF16, False)
    cos_d = nc.declare_dram_parameter("cos2", [128, TPAD], BF16, False)
    sin_d = nc.declare_dram_parameter("sinf2", [128, TPAD], BF16, False)
    idn_d = nc.declare_dram_parameter("ident", [128, 128], BF16, False)
    out_d = nc.declare_dram_parameter("out", [D, N], F32, True)

    with tile.TileContext(nc) as tc:
        with (
            tc.tile_pool(name="const", bufs=1) as cpool,
            tc.tile_pool(name="persist", bufs=1) as ppool,
            tc.tile_pool(name="work", bufs=2) as wpool,
            tc.tile_pool(name="psum", bufs=2, space="PSUM") as pspool,
        ):
            # ---------------- constants ----------------
            wqk_sb = [cpool.tile([128, 512], BF16, tag=f"wqk{c}", name=f"wqk{c}") for c in range(8)]
            wv_sb = [cpool.tile([128, 256], BF16, tag=f"wv{c}", name=f"wv{c}") for c in range(8)]
            wo_sb = [cpool.tile([128, 1024], BF16, tag=f"wo{c}", name=f"wo{c}") for c in range(2)]
            bqk_sb = cpool.tile([1, 512], BF16, tag="bqk", name="bqk")
            bv_sb = cpool.tile([1, 256], BF16, tag="bv", name="bv")
            cos_sb = cpool.tile([128, TPAD], BF16, tag="cos", name="cos")
            sin_sb = cpool.tile([128, TPAD], BF16, tag="sin", name="sin")
            idn_sb = cpool.tile([128, 128], BF16, tag="idn", name="idn")
            ones_sb = cpool.tile([1, TPAD], BF16, tag="ones", name="ones")

            nc.sync.dma_start(out=idn_sb[:], in_=idn_d[:])

            def _load_weights():
                for c in range(8):
                    nc.sync.dma_start(out=wqk_sb[c][:], in_=wqk_d[c * 128:(c + 1) * 128, :])
                    nc.sync.dma_start(out=wv_sb[c][:], in_=wv_d[c * 128:(c + 1) * 128, :])

            def _load_consts():
                for c in range(2):
                    nc.gpsimd.dma_start(out=wo_sb[c][:], in_=wo_d[c * 128:(c + 1) * 128, :])
                nc.gpsimd.dma_start(out=bqk_sb[:], in_=bqk_d[:])
                nc.gpsimd.dma_start(out=bv_sb[:], in_=bv_d[:])
                nc.gpsimd.dma_start(out=cos_sb[:], in_=cos_d[:])
                nc.gpsimd.dma_start(out=sin_sb[:], in_=sin_d[:])

            nc.vector.memset(ones_sb[:], 1.0)
            eps_sb = cpool.tile([128, 1], F32, tag="eps", name="eps")
            nc.vector.memset(eps_sb[:], 1e-5)
            ones64_sb = cpool.tile([128, 64], BF16, tag="ones64", name="ones64")
            nc.vector.memset(ones64_sb[:], 1.0)

            # ---------------- persistent activations ----------------
            # xT[g]: [128, 8 dchunks, tg-width] per token group g (fine-grained deps)
            xTg = [
                ppool.tile([128, 8 * tw], BF16, tag=f"xT{g}", name=f"xT{g}")
                for g, (to, tw) in enumerate(TG)
            ]
            xT3 = [
                xTg[g][:, :].rearrange("p (c t) -> p c t", c=8) for g in range(5)
            ]

            def xslice(c, jo, jw):
                g = _tg_of(jo)
                to, tw = TG[g]
                assert jo + jw <= to + tw
                return xT3[g][:, c, jo - to:jo - to + jw]

            # qkT tiles: 0,1 = q head-pairs (h01, h23); 2,3 = k head-pairs
            qkT_sb = [ppool.tile([128, TPAD], BF16, tag=f"qkT{f}", name=f"qkT{f}") for f in range(4)]
            vaug_sb = [ppool.tile([128, 260], BF16, tag=f"v{k}", name=f"v{k}") for k in range(NT)]

            # ---------------- phase A: LayerNorm + transpose ----------------
            for i in range(NT):
                xa = wpool.tile([128, D], F32, tag="xa", name="xa", bufs=3)
                if i < 16:
                    nc.sync.dma_start(out=xa[:], in_=x_d[i * 128:(i + 1) * 128, :])
                else:
                    nc.vector.memset(xa[:], 0.0)
                    nc.sync.dma_start(out=xa[0:1, :], in_=x_d[2048:2049, :])
                if i == 2:
                    _load_weights()
                if i == 4:
                    _load_consts()
                stats = wpool.tile([128, 12], F32, tag="stats", name="stats", bufs=3)
                mv = wpool.tile([128, 2], F32, tag="mv", name="mv", bufs=4)
                nc.vector.bn_stats(stats[:, 0:6], xa[:, 0:512])
                nc.vector.bn_stats(stats[:, 6:12], xa[:, 512:1024])
                nc.vector.bn_aggr(mv[:], stats[:])
                std = wpool.tile([128, 1], F32, tag="std", name="std", bufs=4)
                rstd = wpool.tile([128, 1], F32, tag="rstd", name="rstd", bufs=4)
                murstd = wpool.tile([128, 1], F32, tag="murstd", name="murstd")
                nc.scalar.activation(std[:], mv[:, 1:2], AF.Sqrt, bias=eps_sb[:])
                nc.vector.reciprocal(rstd[:], std[:])
                nc.vector.tensor_mul(murstd[:], mv[:, 0:1], rstd[:])
                xn = wpool.tile([128, D], BF16, tag="xn", name="xn", bufs=4)
                nc.vector.tensor_scalar(
                    xn[:], xa[:], rstd[:], murstd[:], OP.mult, OP.subtract
                )
                g = _tg_of(i * 128)
                to, tw = TG[g]
                for s in range(2):
                    pst = pspool.tile([128, 512], BF16, tag="dots", name="pst", bufs=3)
                    for c in range(4):
                        nc.tensor.transpose(
                            pst[:, c * 128:(c + 1) * 128],
                            xn[:, (4 * s + c) * 128:(4 * s + c + 1) * 128],
                            idn_sb[:],
                        )
                    nc.vector.tensor_copy(
                        xT3[g][:, 4 * s:4 * s + 4, i * 128 - to:(i + 1) * 128 - to],
                        pst[:, :].rearrange("p (c t) -> p c t", c=4),
                    )

            # ---------------- phase B1: Q,K projection + RoPE ----------------
            for f in (0, 2, 1, 3):
                qf = wpool.tile([128, TPAD], F32, tag="qf", name="qf", bufs=2)
                for jo, jw in J5:
                    psq = pspool.tile([128, 512], F32, tag="po", name="psq", bufs=2)
                    for c in range(8):
                        nc.tensor.matmul(
                            psq[:, 0:jw],
                            wqk_sb[c][:, f * 128:(f + 1) * 128],
                            xslice(c, jo, jw),
                            start=(c == 0),
                            stop=False,
                        )
                    nc.tensor.matmul(
                        psq[:, 0:jw],
                        bqk_sb[:, f * 128:(f + 1) * 128],
                        ones_sb[:, jo:jo + jw],
                        start=False,
                        stop=True,
                    )
                    nc.scalar.copy(qf[:, jo:jo + jw], psq[:, 0:jw])
                # rotate-half partner via partition-shifting DMAs
                qs = wpool.tile([128, TPAD], F32, tag="qs", name="qs", bufs=1)
                nc.sync.dma_start(out=qs[0:32, :], in_=qf[32:64, :])
                nc.sync.dma_start(out=qs[32:64, :], in_=qf[0:32, :])
                nc.sync.dma_start(out=qs[64:96, :], in_=qf[96:128, :])
                nc.sync.dma_start(out=qs[96:128, :], in_=qf[64:96, :])
                for jo, jw in J5:
                    t1 = wpool.tile([128, 512], F32, tag="t1", name="t1", bufs=2)
                    t2 = wpool.tile([128, 512], F32, tag="t2", name="t2", bufs=2)
                    nc.vector.tensor_mul(
                        t1[:, 0:jw], qf[:, jo:jo + jw], cos_sb[:, jo:jo + jw]
                    )
                    nc.vector.tensor_mul(
                        t2[:, 0:jw], qs[:, jo:jo + jw], sin_sb[:, jo:jo + jw]
                    )
                    nc.vector.tensor_add(
                        qkT_sb[f][:, jo:jo + jw], t1[:, 0:jw], t2[:, 0:jw]
                    )

            # ---------------- phase B2: V projection -> v_aug ----------------
            for k in range(NT):
                psv = pspool.tile([128, 256], F32, tag="po", name="psv", bufs=2)
                for c in range(8):
                    nc.tensor.matmul(
                        psv[:],
                        xslice(c, k * 128, 128),
                        wv_sb[c][:],
                        start=(c == 0),
                        stop=False,
                    )
                nc.tensor.matmul(
                    psv[:],
                    ones_sb[:, k * 128:(k + 1) * 128],
                    bv_sb[:],
                    start=False,
                    stop=True,
                )
                va = vaug_sb[k]
                va3 = va[:, :].rearrange("p (a b) -> p a b", a=4)
                if k < 16:
                    nc.scalar.copy(
                        va3[:, :, 0:64], psv[:, :].rearrange("p (a b) -> p a b", a=4)
                    )
                    nc.vector.memset(va3[:, :, 64:65], 1.0)
                else:
                    # only token 2048 is real; zero rows kill padded keys
                    nc.vector.memset(va[:], 0.0)
                    va3r = va[0:1, :].rearrange("p (a b) -> p a b", a=4)
                    nc.scalar.copy(
                        va3r[:, :, 0:64],
                        psv[0:1, :].rearrange("p (a b) -> p a b", a=4),
                    )
                    nc.vector.memset(va3r[:, :, 64:65], 1.0)

            # ---------------- phases C+D: attention, normalize, out-proj ----------------
            # Heads are emitted h0,h1,h2,[pair0 epilogue],h3,[pair1 epilogue],
            # out-proj: pair0's reciprocal chain resolves during h2's attention
            # so the in-order PE queue never stalls on it; even heads write
            # their normalized tile straight into an[] on DVE (partition 0:64).

            def att_head(jo, jw, h, oev_l, dnp):
                qt = qkT_sb[h // 2]
                ktile = qkT_sb[2 + h // 2]
                po = (h % 2) * 64
                pso = [
                    pspool.tile([65, 512], F32, tag="po", name=f"pso{si}", bufs=2)
                    for si in range(2 if jw > 1 else 1)
                ]
                if jw == 1:
                    psd = pspool.tile([128, NT], F32, tag="dots", name="psdt", bufs=3)
                    for k in range(NT):
                        nc.tensor.matmul(
                            psd[:, k:k + 1],
                            ktile[po:po + 64, k * 128:(k + 1) * 128],
                            qt[po:po + 64, jo:jo + 1],
                        )
                    ex = wpool.tile([128, 1024], BF16, tag="ex", name="ex", bufs=4)
                    nc.scalar.activation(ex[:, 0:NT], psd[:], AF.Exp, scale=SCALE)
                    for k in range(NT):
                        nc.tensor.matmul(
                            pso[0][:, 0:1],
                            vaug_sb[k][:, h * 65:h * 65 + 65],
                            ex[:, k:k + 1],
                            start=(k == 0),
                            stop=(k == NT - 1),
                            skip_group_check=True,
                        )
                else:
                    for k in range(NT):
                        psd = pspool.tile([128, 1024], F32, tag="dots", name="psd", bufs=3)
                        for so, sw in _subs(jw):
                            nc.tensor.matmul(
                                psd[:, so:so + sw],
                                ktile[po:po + 64, k * 128:(k + 1) * 128],
                                qt[po:po + 64, jo + so:jo + so + sw],
                            )
                        ex = wpool.tile([128, 1024], BF16, tag="ex", name="ex", bufs=4)
                        nc.scalar.activation(
                            ex[:, 0:jw], psd[:, 0:jw], AF.Exp, scale=SCALE
                        )
                        for si, (so, sw) in enumerate(_subs(jw)):
                            nc.tensor.matmul(
                                pso[si][:, 0:sw],
                                vaug_sb[k][:, h * 65:h * 65 + 65],
                                ex[:, so:so + sw],
                                start=(k == 0),
                                stop=(k == NT - 1),
                                skip_group_check=True,
                            )
                oev = (wpool.tile([65, 64], BF16, tag="oevt", name="oevt", bufs=4)
                       if jw == 1 else
                       wpool.tile([65, 1024], BF16, tag="oev", name="oev", bufs=4))
                for si, (so, sw) in enumerate(_subs(jw)):
                    nc.vector.tensor_copy(oev[:, so:so + sw], pso[si][:, 0:sw])
                oev_l.append(oev)
                nc.gpsimd.dma_start(
                    out=dnp[h // 2][32 * (h % 2):32 * (h % 2) + 1, 0:jw],
                    in_=oev[64:65, 0:jw],
                )

            def epi_pair(jo, jw, p, oev_l, dnp, an):
                dnb = wpool.tile([64, 1024], BF16, tag=f"dnb{p}", name=f"dnb{p}", bufs=2)
                for so, sw in _subs(jw):
                    dnr = wpool.tile([64, 512], F32, tag="dnr", name="dnr", bufs=2)
                    nc.vector.reciprocal(dnr[:, 0:sw], dnp[p][:, so:so + sw])
                    nc.vector.tensor_copy(dnb[:, so:so + sw], dnr[:, 0:sw])
                for h in (2 * p, 2 * p + 1):
                    hr = 32 * (h % 2)
                    for so, sw in _subs(jw):
                        psb = pspool.tile([64, 512], F32, tag="po", name="psb", bufs=2)
                        nc.tensor.matmul(
                            psb[:, 0:sw],
                            ones64_sb[hr:hr + 1, :],
                            dnb[hr:hr + 1, so:so + sw],
                        )
                        if h % 2 == 0:
                            nc.vector.tensor_mul(
                                an[h // 2][0:64, so:so + sw],
                                oev_l[h][0:64, so:so + sw],
                                psb[:, 0:sw],
                            )
                        else:
                            nt = wpool.tile([64, 512], BF16, tag="nt", name="nt", bufs=4)
                            nc.vector.tensor_mul(
                                nt[:, 0:sw], oev_l[h][0:64, so:so + sw], psb[:, 0:sw]
                            )
                            nc.gpsimd.dma_start(
                                out=an[h // 2][64:128, so:so + sw], in_=nt[:, 0:sw]
                            )

            def outproj(jo, jw, an, final=False):
                for of in range(8):
                    for so, sw in _subs(jw):
                        swr = min(sw, max(0, N - (jo + so)))
                        pp = pspool.tile([128, 512], F32, tag="po", name="pp", bufs=2)
                        nc.tensor.matmul(
                            pp[:, 0:sw],
                            wo_sb[0][:, of * 128:(of + 1) * 128],
                            an[0][:, so:so + sw],
                            start=True,
                            stop=False,
                        )
                        nc.tensor.matmul(
                            pp[:, 0:sw],
                            wo_sb[1][:, of * 128:(of + 1) * 128],
                            an[1][:, so:so + sw],
                            start=False,
                            stop=True,
                        )
                        oo = wpool.tile([128, 512], F32, tag="oo", name="oo", bufs=3)
                        if final and of % 2 == 1:
                            nc.scalar.copy(oo[:, 0:swr], pp[:, 0:swr])
                        else:
                            nc.vector.tensor_copy(oo[:, 0:swr], pp[:, 0:swr])
                        nc.sync.dma_start(
                            out=out_d[of * 128:(of + 1) * 128, jo + so:jo + so + swr],
                            in_=oo[:, 0:swr],
                        )

            prev_op = None
            for jo, jw in JJ:
                an = [
                    wpool.tile([128, 1024], BF16, tag=f"an{ch}", name=f"an{ch}", bufs=2)
                    for ch in range(2)
                ]
                dnp = [
                    wpool.tile([64, 1024], BF16, tag=f"dnp{p}", name=f"dnp{p}", bufs=2)
                    for p in range(2)
                ]
                oev_l = []
                att_head(jo, jw, 0, oev_l, dnp)
                att_head(jo, jw, 1, oev_l, dnp)
                att_head(jo, jw, 2, oev_l, dnp)
                epi_pair(jo, jw, 0, oev_l, dnp, an)
                att_head(jo, jw, 3, oev_l, dnp)
                epi_pair(jo, jw, 1, oev_l, dnp, an)
                if prev_op is not None:
                    outproj(*prev_op)
                prev_op = (jo, jw, an)
            outproj(*prev_op, final=True)
    nc.compile()
    return nc


_NC = None


def _get_nc():
    global _NC
    if _NC is None:
        _NC = _build()
    return _NC


def _make_inputs(x, ln_gamma, ln_beta, w_qkv, w_out):
    w_eff = (w_qkv * ln_gamma[:, None].astype(np.float32)).astype(np.float32)
    b_all = (ln_beta.astype(np.float32) @ w_qkv.astype(np.float32)).astype(np.float32)

    inv = 1.0 / (10000.0 ** (np.arange(0, 64, 2, dtype=np.float64) / 64.0))
    fr = np.arange(2048, dtype=np.float64)[:, None] * inv[None, :]
    cos64 = np.concatenate([np.cos(fr), np.cos(fr)], axis=1).T  # [64, 2048]
    sinf64 = np.concatenate([-np.sin(fr), np.sin(fr)], axis=1).T
    cos2 = np.ones((128, TPAD), np.float32)
    sinf2 = np.zeros((128, TPAD), np.float32)
    cos2[:, 1:2049] = np.tile(cos64, (2, 1)).astype(np.float32)
    sinf2[:, 1:2049] = np.tile(sinf64, (2, 1)).astype(np.float32)
    ident = np.eye(128, dtype=BF)

    in_maps = []
    for c in range(8):
        b, g = c // 4, c % 4
        cols = slice(256 * g, 256 * g + 256)
        wqk = np.concatenate(
            [w_eff[:, 0:1024][:, cols], w_eff[:, 1024:2048][:, cols]], axis=1
        ).astype(BF)
        wv = w_eff[:, 2048:3072][:, cols].astype(BF)
        wo = w_out[cols, :].astype(BF)
        bqk = np.concatenate([b_all[0:1024][cols], b_all[1024:2048][cols]])[None, :].astype(BF)
        bv = b_all[2048:3072][cols][None, :].astype(BF)
        in_maps.append(
            {
                "x": np.ascontiguousarray(x[b]).astype(np.float32),
                "wqk": np.ascontiguousarray(wqk),
                "wv": np.ascontiguousarray(wv),
                "wo": np.ascontiguousarray(wo),
                "bqk": bqk,
                "bv": bv,
                "cos2": cos2.astype(BF),
                "sinf2": sinf2.astype(BF),
                "ident": ident,
            }
        )
    return in_maps


def kernel(x, ln_gamma, ln_beta, w_qkv, w_out, _trace=False, _trace_kwargs=None):
    nc = _get_nc()
    in_maps = _make_inputs(x, ln_gamma, ln_beta, w_qkv, w_out)
    res = run_bass_kernel_spmd(
        nc, in_maps, core_ids=list(range(8)), trace=_trace,
        **(_trace_kwargs or {}),
    )
    out = np.zeros((B, N, D), np.float32)
    for c in range(8):
        out[c // 4] += np.asarray(res.results[c]["out"], np.float32).T
    if _trace:
        return out, res
    return out


# revision 40
# speedup vs baseline: 1.2711x; 1.0151x over previous
"""Trainium2 Bass kernel for a ViT attention block (LN -> QKV -> RoPE -> attn -> out-proj).

Sharding: 8 cores = 2 batches x 4 head-groups (4 heads each). Each core computes
a partial out-projection (its 4 heads) for one batch, transposed as [D, N].
Host sums the 4 partials per batch and transposes back. LayerNorm gamma/beta are
folded into the QKV weights/bias on the host.

Device layout (per core):
  - LN in [tok, d] layout (bn_stats per-partition), normalized output cast bf16,
    PE-transposed into xT tiles ([d-chunk, tok] layout, split into 5 token
    groups so the QKV projection can start before LayerNorm finishes).
  - Q,K projected directly into [feat, tok] (lhsT = weight chunks); V projected
    into [tok, feat] (lhsT = xT chunks). QKV bias enters as a K=1 matmul row.
  - RoPE in [feat, tok] with host-built cos/sin tables ([128] rows = 2 heads
    stacked; cls token and padding encoded in the tables). The rotate-half
    partner comes from partition-shifting SBUF->SBUF DMAs.
  - Attention transposed: dotsT[k,q] = matmul(lhsT=kT, rhs=qT), exp on ACT
    (logits are bounded, no max subtraction), attn@v as outT[65,q] with an
    appended ones column in V giving the softmax denominator for free.
  - Denominators are DMA'd into 32-aligned partitions of one tile so a single
    full-width reciprocal+cast serves all 4 heads of a q-chunk; normalize via
    ones outer-product broadcast (PE) + DVE multiply; out-proj accumulates the
    256 head dims; result DMA'd out as [D, N] (transposed).
All matmuls bf16 with f32 PSUM accumulation.
"""

import sys

sys.path.insert(0, "/opt/trn_rl_repo")

import numpy as np
import ml_dtypes

import concourse.bacc as bacc
import concourse.mybir as mybir
import concourse.tile as tile
from concourse.bass_utils import run_bass_kernel_spmd

F32 = mybir.dt.float32
BF16 = mybir.dt.bfloat16
AF = mybir.ActivationFunctionType
OP = mybir.AluOpType
BF = ml_dtypes.bfloat16

B, N, D = 2, 2049, 1024
DH = 64
HPC = 4  # heads per core
NT = 17  # 128-token tiles (padded to 2176)
TPAD = NT * 128
SCALE = DH ** -0.5
# q-column chunks [offset, width]; the tail chunk is the single real token 2048
JJ = [(2048, 1), (0, 1024), (1024, 1024)]
J5 = [(0, 512), (512, 512), (1024, 512), (1536, 512), (2048, 128)]
# token groups backing the 5 xT tiles (4+4+4+4+1 of the 17 token tiles)
TG = [(0, 512), (512, 512), (1024, 512), (1536, 512), (2048, 128)]


def _subs(jw):
    return [(s, min(512, jw - s)) for s in range(0, jw, 512)]


def _tg_of(col):
    return min(col // 512, 4)


def _build():
    nc = bacc.Bacc("TRN2", target_bir_lowering=False, debug=False, num_devices=8)

    x_d = nc.declare_dram_parameter("x", [N, D], F32, False)
    wqk_d = nc.declare_dram_parameter("wqk", [D, 512], BF16, False)
    wv_d = nc.declare_dram_parameter("wv", [D, 256], BF16, False)
    wo_d = nc.declare_dram_parameter("wo", [256, D], BF16, False)
    bqk_d = nc.declare_dram_parameter("bqk", [1, 512], BF16, False)
    bv_d = nc.declare_dram_parameter("bv", [1, 256], BF16, False)
    cos_d = nc.declare_dram_parameter("cos2", [128, TPAD], BF16, False)
    sin_d = nc.declare_dram_parameter("sinf2", [128, TPAD], BF16, False)
    idn_d = nc.declare_dram_parameter("ident", [128, 128], BF16, False)
    out_d = nc.declare_dram_parameter("out", [D, N], F32, True)

    with tile.TileContext(nc) as tc:
        with (
            tc.tile_pool(name="const", bufs=1) as cpool,
            tc.tile_pool(name="persist", bufs=1) as ppool,
            tc.tile_pool(name="work", bufs=2) as wpool,
            tc.tile_pool(name="psum", bufs=2, space="PSUM") as pspool,
        ):
            # ---------------- constants ----------------
            wqk_sb = [cpool.tile([128, 512], BF16, tag=f"wqk{c}", name=f"wqk{c}") for c in range(8)]
            wv_sb = [cpool.tile([128, 256], BF16, tag=f"wv{c}", name=f"wv{c}") for c in range(8)]
            wo_sb = [cpool.tile([128, 1024], BF16, tag=f"wo{c}", name=f"wo{c}") for c in range(2)]
            bqk_sb = cpool.tile([1, 512], BF16, tag="bqk", name="bqk")
            bv_sb = cpool.tile([1, 256], BF16, tag="bv", name="bv")
            cos_sb = cpool.tile([128, TPAD], BF16, tag="cos", name="cos")
            sin_sb = cpool.tile([128, TPAD], BF16, tag="sin", name="sin")
            idn_sb = cpool.tile([128, 128], BF16, tag="idn", name="idn")
            ones_sb = cpool.tile([1, TPAD], BF16, tag="ones", name="ones")

            nc.sync.dma_start(out=idn_sb[:], in_=idn_d[:])

            def _load_weights():
                for c in range(8):
                    nc.sync.dma_start(out=wqk_sb[c][:], in_=wqk_d[c * 128:(c + 1) * 128, :])
                    nc.sync.dma_start(out=wv_sb[c][:], in_=wv_d[c * 128:(c + 1) * 128, :])

            def _load_consts():
                for c in range(2):
                    nc.gpsimd.dma_start(out=wo_sb[c][:], in_=wo_d[c * 128:(c + 1) * 128, :])
                nc.gpsimd.dma_start(out=bqk_sb[:], in_=bqk_d[:])
                nc.gpsimd.dma_start(out=bv_sb[:], in_=bv_d[:])
                nc.gpsimd.dma_start(out=cos_sb[:], in_=cos_d[:])
                nc.gpsimd.dma_start(out=sin_sb[:], in_=sin_d[:])

            nc.vector.memset(ones_sb[:], 1.0)
            eps_sb = cpool.tile([128, 1], F32, tag="eps", name="eps")
            nc.vector.memset(eps_sb[:], 1e-5)
            ones64_sb = cpool.tile([128, 64], BF16, tag="ones64", name="ones64")
            nc.vector.memset(ones64_sb[:], 1.0)

            # ---------------- persistent activations ----------------
            # xT[g]: [128, 8 dchunks, tg-width] per token group g (fine-grained deps)
            xTg = [
                ppool.tile([128, 8 * tw], BF16, tag=f"xT{g}", name=f"xT{g}")
                for g, (to, tw) in enumerate(TG)
            ]
            xT3 = [
                xTg[g][:, :].rearrange("p (c t) -> p c t", c=8) for g in range(5)
            ]

            def xslice(c, jo, jw):
                g = _tg_of(jo)
                to, tw = TG[g]
                assert jo + jw <= to + tw
                return xT3[g][:, c, jo - to:jo - to + jw]

            # qkT tiles: 0,1 = q head-pairs (h01, h23); 2,3 = k head-pairs
            qkT_sb = [ppool.tile([128, TPAD], BF16, tag=f"qkT{f}", name=f"qkT{f}") for f in range(4)]
            vaug_sb = [ppool.tile([128, 260], BF16, tag=f"v{k}", name=f"v{k}") for k in range(NT)]

            # ---------------- phase A: LayerNorm + transpose ----------------
            for i in range(NT):
                xa = wpool.tile([128, D], F32, tag="xa", name="xa", bufs=3)
                if i < 16:
                    nc.sync.dma_start(out=xa[:], in_=x_d[i * 128:(i + 1) * 128, :])
                else:
                    nc.vector.memset(xa[:], 0.0)
                    nc.sync.dma_start(out=xa[0:1, :], in_=x_d[2048:2049, :])
                if i == 2:
                    _load_weights()
                if i == 4:
                    _load_consts()
                stats = wpool.tile([128, 12], F32, tag="stats", name="stats", bufs=3)
                mv = wpool.tile([128, 2], F32, tag="mv", name="mv", bufs=4)
                nc.vector.bn_stats(stats[:, 0:6], xa[:, 0:512])
                nc.vector.bn_stats(stats[:, 6:12], xa[:, 512:1024])
                nc.vector.bn_aggr(mv[:], stats[:])
                std = wpool.tile([128, 1], F32, tag="std", name="std", bufs=4)
                rstd = wpool.tile([128, 1], F32, tag="rstd", name="rstd", bufs=4)
                murstd = wpool.tile([128, 1], F32, tag="murstd", name="murstd")
                nc.scalar.activation(std[:], mv[:, 1:2], AF.Sqrt, bias=eps_sb[:])
                nc.vector.reciprocal(rstd[:], std[:])
                nc.vector.tensor_mul(murstd[:], mv[:, 0:1], rstd[:])
                xn = wpool.tile([128, D], BF16, tag="xn", name="xn", bufs=4)
                nc.vector.tensor_scalar(
                    xn[:], xa[:], rstd[:], murstd[:], OP.mult, OP.subtract
                )
                g = _tg_of(i * 128)
                to, tw = TG[g]
                for s in range(2):
                    pst = pspool.tile([128, 512], BF16, tag="dots", name="pst", bufs=3)
                    for c in range(4):
                        nc.tensor.transpose(
                            pst[:, c * 128:(c + 1) * 128],
                            xn[:, (4 * s + c) * 128:(4 * s + c + 1) * 128],
                            idn_sb[:],
                        )
                    nc.vector.tensor_copy(
                        xT3[g][:, 4 * s:4 * s + 4, i * 128 - to:(i + 1) * 128 - to],
                        pst[:, :].rearrange("p (c t) -> p c t", c=4),
                    )

            # ---------------- phase B1: Q,K projection + RoPE ----------------
            for f in (0, 2, 1, 3):
                qf = wpool.tile([128, TPAD], F32, tag="qf", name="qf", bufs=2)
                for jo, jw in J5:
                    psq = pspool.tile([128, 512], F32, tag="po", name="psq", bufs=2)
                    for c in range(8):
                        nc.tensor.matmul(
                            psq[:, 0:jw],
                            wqk_sb[c][:, f * 128:(f + 1) * 128],
                            xslice(c, jo, jw),
                            start=(c == 0),
                            stop=False,
                        )
                    nc.tensor.matmul(
                        psq[:, 0:jw],
                        bqk_sb[:, f * 128:(f + 1) * 128],
                        ones_sb[:, jo:jo + jw],
                        start=False,
                        stop=True,
                    )
                    nc.scalar.copy(qf[:, jo:jo + jw], psq[:, 0:jw])
                # rotate-half partner via partition-shifting DMAs
                qs = wpool.tile([128, TPAD], F32, tag="qs", name="qs", bufs=1)
                nc.sync.dma_start(out=qs[0:32, :], in_=qf[32:64, :])
                nc.sync.dma_start(out=qs[32:64, :], in_=qf[0:32, :])
                nc.sync.dma_start(out=qs[64:96, :], in_=qf[96:128, :])
                nc.sync.dma_start(out=qs[96:128, :], in_=qf[64:96, :])
                for jo, jw in J5:
                    t1 = wpool.tile([128, 512], F32, tag="t1", name="t1", bufs=2)
                    t2 = wpool.tile([128, 512], F32, tag="t2", name="t2", bufs=2)
                    nc.vector.tensor_mul(
                        t1[:, 0:jw], qf[:, jo:jo + jw], cos_sb[:, jo:jo + jw]
                    )
                    nc.vector.tensor_mul(
                        t2[:, 0:jw], qs[:, jo:jo + jw], sin_sb[:, jo:jo + jw]
                    )
                    nc.vector.tensor_add(
                        qkT_sb[f][:, jo:jo + jw], t1[:, 0:jw], t2[:, 0:jw]
                    )

            # ---------------- phase B2: V projection -> v_aug ----------------
            for k in range(NT):
                psv = pspool.tile([128, 256], F32, tag="po", name="psv", bufs=2)
                for c in range(8):
                    nc.tensor.matmul(
                        psv[:],
                        xslice(c, k * 128, 128),
                        wv_sb[c][:],
                        start=(c == 0),
                        stop=False,
                    )
                nc.tensor.matmul(
                    psv[:],
                    ones_sb[:, k * 128:(k + 1) * 128],
                    bv_sb[:],
                    start=False,
                    stop=True,
                )
                va = vaug_sb[k]
                va3 = va[:, :].rearrange("p (a b) -> p a b", a=4)
                if k < 16:
                    nc.scalar.copy(
                        va3[:, :, 0:64], psv[:, :].rearrange("p (a b) -> p a b", a=4)
                    )
                    nc.vector.memset(va3[:, :, 64:65], 1.0)
                else:
                    # only token 2048 is real; zero rows kill padded keys
                    nc.vector.memset(va[:], 0.0)
                    va3r = va[0:1, :].rearrange("p (a b) -> p a b", a=4)
                    nc.scalar.copy(
                        va3r[:, :, 0:64],
                        psv[0:1, :].rearrange("p (a b) -> p a b", a=4),
                    )
                    nc.vector.memset(va3r[:, :, 64:65], 1.0)

            # ---------------- phases C+D: attention, normalize, out-proj ----------------
            # Heads are emitted h0,h1,h2,[pair0 epilogue],h3,[pair1 epilogue],
            # out-proj: pair0's reciprocal chain resolves during h2's attention
            # so the in-order PE queue never stalls on it; even heads write
            # their normalized tile straight into an[] on DVE (partition 0:64).

            def att_head(jo, jw, h, oev_l, dnp, side=None):
                qt = qkT_sb[h // 2]
                ktile = qkT_sb[2 + h // 2]
                po = (h % 2) * 64
                pso = [
                    pspool.tile([65, 512], F32, tag="po", name=f"pso{si}", bufs=2)
                    for si in range(2 if jw > 1 else 1)
                ]
                if jw == 1:
                    psd = pspool.tile([128, NT], F32, tag="dots", name="psdt", bufs=3)
                    for k in range(NT):
                        nc.tensor.matmul(
                            psd[:, k:k + 1],
                            ktile[po:po + 64, k * 128:(k + 1) * 128],
                            qt[po:po + 64, jo:jo + 1],
                        )
                    ex = wpool.tile([128, 1024], BF16, tag="ex", name="ex", bufs=4)
                    nc.scalar.activation(ex[:, 0:NT], psd[:], AF.Exp, scale=SCALE)
                    for k in range(NT):
                        nc.tensor.matmul(
                            pso[0][:, 0:1],
                            vaug_sb[k][:, h * 65:h * 65 + 65],
                            ex[:, k:k + 1],
                            start=(k == 0),
                            stop=(k == NT - 1),
                            skip_group_check=True,
                        )
                else:
                    for k in range(NT):
                        psd = pspool.tile([128, 1024], F32, tag="dots", name="psd", bufs=3)
                        for so, sw in _subs(jw):
                            nc.tensor.matmul(
                                psd[:, so:so + sw],
                                ktile[po:po + 64, k * 128:(k + 1) * 128],
                                qt[po:po + 64, jo + so:jo + so + sw],
                            )
                        ex = wpool.tile([128, 1024], BF16, tag="ex", name="ex", bufs=4)
                        nc.scalar.activation(
                            ex[:, 0:jw], psd[:, 0:jw], AF.Exp, scale=SCALE
                        )
                        for si, (so, sw) in enumerate(_subs(jw)):
                            nc.tensor.matmul(
                                pso[si][:, 0:sw],
                                vaug_sb[k][:, h * 65:h * 65 + 65],
                                ex[:, so:so + sw],
                                start=(k == 0),
                                stop=(k == NT - 1),
                                skip_group_check=True,
                            )
                        if side is not None and k % 4 == 3 and side:
                            emit_op_chunk(*side.pop(0))
                oev = (wpool.tile([65, 64], BF16, tag="oevt", name="oevt", bufs=4)
                       if jw == 1 else
                       wpool.tile([65, 1024], BF16, tag="oev", name="oev", bufs=4))
                for si, (so, sw) in enumerate(_subs(jw)):
                    nc.vector.tensor_copy(oev[:, so:so + sw], pso[si][:, 0:sw])
                oev_l.append(oev)
                nc.gpsimd.dma_start(
                    out=dnp[h // 2][32 * (h % 2):32 * (h % 2) + 1, 0:jw],
                    in_=oev[64:65, 0:jw],
                )

            def epi_pair(jo, jw, p, oev_l, dnp, an):
                dnb = wpool.tile([64, 1024], BF16, tag=f"dnb{p}", name=f"dnb{p}", bufs=2)
                for so, sw in _subs(jw):
                    dnr = wpool.tile([64, 512], F32, tag="dnr", name="dnr", bufs=2)
                    nc.vector.reciprocal(dnr[:, 0:sw], dnp[p][:, so:so + sw])
                    nc.vector.tensor_copy(dnb[:, so:so + sw], dnr[:, 0:sw])
                for h in (2 * p, 2 * p + 1):
                    hr = 32 * (h % 2)
                    for so, sw in _subs(jw):
                        psb = pspool.tile([64, 512], F32, tag="po", name="psb", bufs=2)
                        nc.tensor.matmul(
                            psb[:, 0:sw],
                            ones64_sb[hr:hr + 1, :],
                            dnb[hr:hr + 1, so:so + sw],
                        )
                        if h % 2 == 0:
                            nc.vector.tensor_mul(
                                an[h // 2][0:64, so:so + sw],
                                oev_l[h][0:64, so:so + sw],
                                psb[:, 0:sw],
                            )
                        else:
                            nt = wpool.tile([64, 512], BF16, tag="nt", name="nt", bufs=4)
                            nc.vector.tensor_mul(
                                nt[:, 0:sw], oev_l[h][0:64, so:so + sw], psb[:, 0:sw]
                            )
                            nc.gpsimd.dma_start(
                                out=an[h // 2][64:128, so:so + sw], in_=nt[:, 0:sw]
                            )

            def emit_op_chunk(jo, jw, an, of, so, sw, final=False):
                swr = min(sw, max(0, N - (jo + so)))
                pp = pspool.tile([128, 512], F32, tag="dots", name="pp", bufs=3)
                nc.tensor.matmul(
                    pp[:, 0:sw],
                    wo_sb[0][:, of * 128:(of + 1) * 128],
                    an[0][:, so:so + sw],
                    start=True,
                    stop=False,
                )
                nc.tensor.matmul(
                    pp[:, 0:sw],
                    wo_sb[1][:, of * 128:(of + 1) * 128],
                    an[1][:, so:so + sw],
                    start=False,
                    stop=True,
                )
                oo = wpool.tile([128, 512], F32, tag="oo", name="oo", bufs=3)
                if final and of % 2 == 1:
                    nc.scalar.copy(oo[:, 0:swr], pp[:, 0:swr])
                else:
                    nc.vector.tensor_copy(oo[:, 0:swr], pp[:, 0:swr])
                nc.sync.dma_start(
                    out=out_d[of * 128:(of + 1) * 128, jo + so:jo + so + swr],
                    in_=oo[:, 0:swr],
                )

            def outproj(jo, jw, an, final=False):
                for of in range(8):
                    for so, sw in _subs(jw):
                        emit_op_chunk(jo, jw, an, of, so, sw, final)

            def _unused(jo, jw, an, final=False):
                for of in range(8):
                    for so, sw in _subs(jw):
                        swr = min(sw, max(0, N - (jo + so)))
                        pp = pspool.tile([128, 512], F32, tag="po", name="pp", bufs=2)
                        nc.tensor.matmul(
                            pp[:, 0:sw],
                            wo_sb[0][:, of * 128:(of + 1) * 128],
                            an[0][:, so:so + sw],
                            start=True,
                            stop=False,
                        )
                        nc.tensor.matmul(
                            pp[:, 0:sw],
                            wo_sb[1][:, of * 128:(of + 1) * 128],
                            an[1][:, so:so + sw],
                            start=False,
                            stop=True,
                        )
                        oo = wpool.tile([128, 512], F32, tag="oo", name="oo", bufs=3)
                        if final and of % 2 == 1:
                            nc.scalar.copy(oo[:, 0:swr], pp[:, 0:swr])
                        else:
                            nc.vector.tensor_copy(oo[:, 0:swr], pp[:, 0:swr])
                        nc.sync.dma_start(
                            out=out_d[of * 128:(of + 1) * 128, jo + so:jo + so + swr],
                            in_=oo[:, 0:swr],
                        )

            side = []
            prev = None
            for jo, jw in JJ:
                an = [
                    wpool.tile([128, 1024], BF16, tag=f"an{ch}", name=f"an{ch}", bufs=2)
                    for ch in range(2)
                ]
                dnp = [
                    wpool.tile([64, 1024], BF16, tag=f"dnp{p}", name=f"dnp{p}", bufs=2)
                    for p in range(2)
                ]
                oev_l = []
                sw_arg = side if jw > 1 else None
                att_head(jo, jw, 0, oev_l, dnp, sw_arg)
                att_head(jo, jw, 1, oev_l, dnp, sw_arg)
                att_head(jo, jw, 2, oev_l, dnp, sw_arg)
                epi_pair(jo, jw, 0, oev_l, dnp, an)
                att_head(jo, jw, 3, oev_l, dnp, sw_arg)
                epi_pair(jo, jw, 1, oev_l, dnp, an)
                while side:
                    emit_op_chunk(*side.pop(0))
                side = [
                    (jo, jw, an, of, so, sw)
                    for of in range(8)
                    for so, sw in _subs(jw)
                ]
                prev = (jo, jw, an)
            for (jo_, jw_, an_, of_, so_, sw_) in side:
                emit_op_chunk(jo_, jw_, an_, of_, so_, sw_, final=True)
    nc.compile()
    return nc


_NC = None


def _get_nc():
    global _NC
    if _NC is None:
        _NC = _build()
    return _NC


def _make_inputs(x, ln_gamma, ln_beta, w_qkv, w_out):
    w_eff = (w_qkv * ln_gamma[:, None].astype(np.float32)).astype(np.float32)
    b_all = (ln_beta.astype(np.float32) @ w_qkv.astype(np.float32)).astype(np.float32)

    inv = 1.0 / (10000.0 ** (np.arange(0, 64, 2, dtype=np.float64) / 64.0))
    fr = np.arange(2048, dtype=np.float64)[:, None] * inv[None, :]
    cos64 = np.concatenate([np.cos(fr), np.cos(fr)], axis=1).T  # [64, 2048]
    sinf64 = np.concatenate([-np.sin(fr), np.sin(fr)], axis=1).T
    cos2 = np.ones((128, TPAD), np.float32)
    sinf2 = np.zeros((128, TPAD), np.float32)
    cos2[:, 1:2049] = np.tile(cos64, (2, 1)).astype(np.float32)
    sinf2[:, 1:2049] = np.tile(sinf64, (2, 1)).astype(np.float32)
    ident = np.eye(128, dtype=BF)

    in_maps = []
    for c in range(8):
        b, g = c // 4, c % 4
        cols = slice(256 * g, 256 * g + 256)
        wqk = np.concatenate(
            [w_eff[:, 0:1024][:, cols], w_eff[:, 1024:2048][:, cols]], axis=1
        ).astype(BF)
        wv = w_eff[:, 2048:3072][:, cols].astype(BF)
        wo = w_out[cols, :].astype(BF)
        bqk = np.concatenate([b_all[0:1024][cols], b_all[1024:2048][cols]])[None, :].astype(BF)
        bv = b_all[2048:3072][cols][None, :].astype(BF)
        in_maps.append(
            {
                "x": np.ascontiguousarray(x[b]).astype(np.float32),
                "wqk": np.ascontiguousarray(wqk),
                "wv": np.ascontiguousarray(wv),
                "wo": np.ascontiguousarray(wo),
                "bqk": bqk,
                "bv": bv,
                "cos2": cos2.astype(BF),
                "sinf2": sinf2.astype(BF),
                "ident": ident,
            }
        )
    return in_maps


def kernel(x, ln_gamma, ln_beta, w_qkv, w_out, _trace=False, _trace_kwargs=None):
    nc = _get_nc()
    in_maps = _make_inputs(x, ln_gamma, ln_beta, w_qkv, w_out)
    res = run_bass_kernel_spmd(
        nc, in_maps, core_ids=list(range(8)), trace=_trace,
        **(_trace_kwargs or {}),
    )
    out = np.zeros((B, N, D), np.float32)
    for c in range(8):
        out[c // 4] += np.asarray(res.results[c]["out"], np.float32).T
    if _trace:
        return out, res
    return out
